# revision 13
# baseline (speedup 1.0000x reference)
"""Trainium2 Bass kernel for MinibatchDiscrimination.

Reference op:
    h = (x @ w).reshape(B, U, O)                      # B=512, U=32, O=32
    D[i, o, j] = sum_u |h[i,u,o] - h[j,u,o]|          # pairwise L1 over units
    out[i, o]  = sum_j exp(-D[i,o,j])

Strategy (8 NeuronCores, data-parallel over query rows i, half-pair windows):
  - Host: transpose x -> xT [2048, 512], cast x/w to bf16. Each core c gets
    xT rolled so that its own 64 query columns come first; every core sees
    all 512 comparison columns.
  - Each unordered pair is computed once: query i compares against the 256
    columns [i+1, i+256] (mod 512, wrap-free via column-duplicated tiles).
    The diagonal exp(0)=1 is added on the host. Every computed pair (i,j)
    contributes to F[i] via the in-instruction row accumulation and to F[j]
    via a transposed bf16 accumulator F_colT (all its values are < 1e-7, so
    bf16 is ample). Antipodal pairs (distance 256) are computed from both
    ends; their exp is ~1e-20, invisible in fp32.
  - abs-free L1 via |d| = 2*relu(d) - d, distributed over the unit-sum:
        D[o,j] = 2*sum_u Sel*relu(h_j - h_i) - S[o,j] + S[o,i],
    S[o,j] = sum_u h[j,u,o] (computed once by the same selector matmul).
    The -S[o,j] term rides the SAME stationary matrix sel2 as the relu
    chunks via rhs Sq4 (= -S/2 on partitions 0:32, zeros elsewhere), so all
    phase-2 matmuls share one lhsT; followers in each PSUM accumulation
    chain set ldweights=False to skip redundant PE weight loads. +S[o,i] is
    the per-partition bias of the fused ACT exp+accumulate instruction.
  - Relu chunks are split between DVE (fused tensor_scalar add+max, 2x
    perf mode) and ACT (activation Relu with bias); Relu/Exp/Copy live in
    one ACT table set, so no table reloads.
"""

import os
import sys

import numpy as np

for _p in ("/opt/trn_rl_repo", "/root/.axon_site/_ro/trn_rl_repo"):
    if os.path.isdir(_p) and _p not in sys.path:
        sys.path.insert(0, _p)

import ml_dtypes  # noqa: E402

B = 512  # batch
D = 2048  # in features
U = 32  # units
O = 32  # units_out
UO = U * O  # 1024
NCORES = 8
BL = B // NCORES  # 64 own queries per core
W = 256  # comparison window width (half of B)
BD = B + W  # duplicated-column width (wrap-free windows)

KCH = D // 128  # 16 k-chunks
MCH = UO // 128  # 8 uo-chunks

ACT_EVERY = 4  # every ACT_EVERY-th relu chunk goes to ACT, rest to DVE

_CACHE = {}
LAST_RESULTS = None  # BassKernelResults of the most recent run (for profiling)


def _build():
    """Build + compile the (single, SPMD-identical) Bass program."""
    if "nc" in _CACHE:
        return _CACHE["nc"]

    from contextlib import ExitStack

    import concourse.mybir as mybir
    import concourse.tile as tile
    from concourse import bacc

    bf16 = mybir.dt.bfloat16
    f32 = mybir.dt.float32

    nc = bacc.Bacc(
        "TRN2",
        target_bir_lowering=False,
        debug=False,
        enable_asserts=False,
    )

    xt_d = nc.dram_tensor("xt", [D, B], bf16, kind="ExternalInput")
    w_d = nc.dram_tensor("w", [D, UO], bf16, kind="ExternalInput")
    # sel cols 0:32 = Sel1 (p%32==o), 32:64 = Sel2 = 2*Sel1
    sel_d = nc.dram_tensor("sel", [128, 2 * O], bf16, kind="ExternalInput")
    frow_d = nc.dram_tensor("frow", [O, BL], f32, kind="ExternalOutput")
    fcol_d = nc.dram_tensor("fcol", [O, BD], bf16, kind="ExternalOutput")

    with tile.TileContext(nc) as tc, ExitStack() as ctx:
        persist = ctx.enter_context(tc.tile_pool(name="persist", bufs=1))
        a_pool = ctx.enter_context(tc.tile_pool(name="a", bufs=10))
        e_pool = ctx.enter_context(tc.tile_pool(name="e", bufs=4))
        ph_pool = ctx.enter_context(tc.tile_pool(name="ph", bufs=2, space="PSUM"))
        ps_pool = ctx.enter_context(tc.tile_pool(name="ps", bufs=1, space="PSUM"))
        pd_pool = ctx.enter_context(tc.tile_pool(name="pd", bufs=5, space="PSUM"))

        # --- persistent tiles ---
        sel_sb = persist.tile([128, 2 * O], bf16, tag="sel")
        nc.sync.dma_start(sel_sb[:], sel_d[:])
        sel1 = sel_sb[:, 0:O]

        w_sb = []
        xt_sb = []
        for k in range(KCH):
            wt = persist.tile([128, UO], bf16, tag=f"w{k}", name=f"w{k}")
            nc.sync.dma_start(wt[:], w_d[k * 128 : (k + 1) * 128, :])
            w_sb.append(wt)
            xtt = persist.tile([128, B], bf16, tag=f"xt{k}", name=f"xt{k}")
            nc.sync.dma_start(xtt[:], xt_d[k * 128 : (k + 1) * 128, :])
            xt_sb.append(xtt)

        hT = [
            persist.tile([128, BD], bf16, tag=f"hT{m}", name=f"hT{m}")
            for m in range(MCH)
        ]
        hTneg = [
            persist.tile([128, BL], f32, tag=f"hn{m}", name=f"hn{m}")
            for m in range(MCH)
        ]
        F = persist.tile([O, BL], f32, tag="F")
        FcolT = persist.tile([O, BD], bf16, tag="FcolT")
        Sq4 = persist.tile([128, BD], bf16, tag="Sq4")
        Sneg = persist.tile([O, BL], f32, tag="Sneg")
        sel2_t = persist.tile([128, O], bf16, tag="sel2t")
        zero_col = persist.tile([128, 1], f32, tag="zc")

        nc.gpsimd.memset(FcolT[:], 0.0)
        nc.gpsimd.memset(Sq4[:], 0.0)

        # --- phase 1: hT = (x @ w)^T in bf16, chunked over uo ---
        for m in range(MCH):
            ph = ph_pool.tile([128, B], f32)
            for k in range(KCH):
                nc.tensor.matmul(
                    ph[:],
                    w_sb[k][:, m * 128 : (m + 1) * 128],
                    xt_sb[k][:],
                    start=(k == 0),
                    stop=(k == KCH - 1),
                )
            # PSUM -> SBUF as bf16 (Copy is in the exp/relu table set)
            nc.scalar.activation(hT[m][:, 0:B], ph[:], mybir.ActivationFunctionType.Copy)
            # duplicate first W columns for wrap-free windows
            nc.sync.dma_start(hT[m][:, B:BD], hT[m][:, 0:W])
            # negated f32 bias columns for this core's own queries
            nc.vector.tensor_scalar_mul(hTneg[m][:], hT[m][:, 0:BL], -1.0)

        # --- phase 1b: S[o, j] = sum_u h[j, u, o] once via Sel1 ---
        ps_s = ps_pool.tile([O, B], f32, name="ps_s")
        for m in range(MCH):
            nc.tensor.matmul(
                ps_s[:], sel1, hT[m][:, 0:B], start=(m == 0), stop=(m == MCH - 1)
            )
        # Sq4[0:32] = -S/2 (so sel2 x Sq4 contributes -S[o,j]); rows 32:127 zero
        nc.scalar.activation(
            Sq4[0:O, 0:B], ps_s[:], mybir.ActivationFunctionType.Copy, scale=-0.5
        )
        nc.sync.dma_start(Sq4[0:O, B:BD], Sq4[0:O, 0:W])
        nc.vector.tensor_scalar_mul(Sneg[:], ps_s[:, 0:BL], -1.0)

        # Dependency gate: sel2_t is derived through zero_col <- Sq4 <- ps_s
        # <- all S matmuls <- all hT copies <- all h matmuls. Every phase-2
        # matmul reads sel2_t, so no differently-weighted matmul can be
        # scheduled into phase 2 (required for the ldweights=False skips).
        nc.vector.tensor_scalar(
            zero_col[:], Sq4[:, 0:1], 0.0, None, mybir.AluOpType.mult
        )
        nc.vector.tensor_scalar(
            sel2_t[:], sel_sb[:, O : 2 * O], zero_col[:], None, mybir.AluOpType.add
        )

        # --- phase 2: per-query windowed pairwise L1 + exp-sum ---
        for i in range(BL):
            lo = i + 1  # window = local columns [i+1, i+256]
            pd = pd_pool.tile([O, W], f32)
            mms = []
            for m in range(MCH):
                a = a_pool.tile([128, W], bf16, tag="a")
                if (i * MCH + m) % ACT_EVERY == ACT_EVERY - 1:
                    nc.scalar.activation(
                        a[:],
                        hT[m][:, lo : lo + W],
                        mybir.ActivationFunctionType.Relu,
                        bias=hTneg[m][:, i : i + 1],
                        scale=1.0,
                    )
                else:
                    nc.vector.tensor_scalar(
                        a[:],
                        hT[m][:, lo : lo + W],
                        hTneg[m][:, i : i + 1],
                        0.0,
                        mybir.AluOpType.add,
                        mybir.AluOpType.max,
                    )
                mms.append(
                    nc.tensor.matmul(pd[:], sel2_t[:], a[:], start=(m == 0), stop=False)
                )
            mms.append(
                nc.tensor.matmul(
                    pd[:], sel2_t[:], Sq4[:, lo : lo + W], start=False, stop=True
                )
            )
            # Followers of each accumulation chain reuse the loaded sel2_t
            # (chain order is enforced by the shared PSUM bank).
            for bi in mms[1:]:
                bi.ins.ldweights = False

            e = e_pool.tile([O, W], bf16, tag="e")
            nc.scalar.activation(
                e[:],
                pd[:],
                mybir.ActivationFunctionType.Exp,
                bias=Sneg[:, i : i + 1],
                scale=-1.0,
                accum_out=F[:, i : i + 1],
            )
            # transposed-side contributions (tiny values; bf16 is ample)
            nc.vector.tensor_tensor(
                FcolT[:, lo : lo + W], FcolT[:, lo : lo + W], e[:], mybir.AluOpType.add
            )

        nc.sync.dma_start(frow_d[:], F[:])
        nc.sync.dma_start(fcol_d[:], FcolT[:])

    nc.compile()
    _strip_redundant_ldweights(nc)
    _CACHE["nc"] = nc
    return nc


def _strip_redundant_ldweights(nc):
    """Drop PE weight reloads whose weights AP matches the already-loaded one.

    The Tile lowering splits every matmul into Ldweights+Matmult (matmuls all
    carry ldweights=False). Phase 2 issues 576 matmuls with the same
    stationary matrix; reloading it each time costs ~35us of PE. A reload is
    removable iff it has no semaphore waits/updates and the previous PE
    weight load targeted the identical AP; any unrecognized PE instruction
    conservatively invalidates the tracked state.
    """
    import concourse.mybir as mybir

    PE = mybir.EngineType.PE
    keep_state = {"InstMatmult", "InstDrain", "InstEventSemaphore", "InstNop"}
    removed = 0
    for blk in nc.m.functions[0].blocks:
        insts = blk.instructions
        out = []
        last_w = None
        for inst in insts:
            nm = type(inst).__name__
            if nm == "InstLdweights":
                ap = inst.ins[0]
                key = (
                    ap.memref,
                    ap.offset,
                    tuple(map(tuple, ap.ap)),
                    str(ap.dtype),
                    inst.is_transpose,
                    inst.perf_mode,
                )
                si = inst.sync_info
                has_sync = si is not None and (
                    list(si.on_wait or []) or list(si.on_update or [])
                )
                if not has_sync and key == last_w:
                    removed += 1
                    continue
                last_w = key
            elif nm not in keep_state and getattr(inst, "engine", None) == PE:
                last_w = None
            out.append(inst)
        if removed:
            blk.instructions = out
    return removed


def _make_inputs(x: np.ndarray, w: np.ndarray):
    """Host-side prep: transpose/cast/roll into per-core input maps."""
    xt = np.ascontiguousarray(x.T).astype(ml_dtypes.bfloat16)  # [D, B]
    wb = w.astype(ml_dtypes.bfloat16)  # [D, UO]
    sel = np.zeros((128, 2 * O), dtype=ml_dtypes.bfloat16)
    sel[np.arange(128), np.arange(128) % O] = 1
    sel[np.arange(128), O + np.arange(128) % O] = 2
    in_maps = []
    for c in range(NCORES):
        xt_c = np.roll(xt, -BL * c, axis=1)
        in_maps.append({"xt": np.ascontiguousarray(xt_c), "w": wb, "sel": sel})
    return in_maps


def _assemble(results) -> np.ndarray:
    """Host-side gather: diagonal + row accums + transposed col accums."""
    out = np.ones((B, O), dtype=np.float64)
    for c in range(NCORES):
        frow = np.asarray(results[c]["frow"]).astype(np.float64)  # [O, BL]
        out[BL * c : BL * (c + 1), :] += frow.T
        fcol = np.asarray(results[c]["fcol"]).astype(np.float64)  # [O, BD]
        fold = fcol[:, :B].copy()
        fold[:, :W] += fcol[:, B:BD]
        idx = (np.arange(B) + BL * c) % B
        out[idx, :] += fold.T
    return out.astype(np.float32)


def kernel(x: np.ndarray, w: np.ndarray) -> np.ndarray:
    global LAST_RESULTS
    from concourse.bass_utils import run_bass_kernel_spmd

    nc = _build()
    in_maps = _make_inputs(np.asarray(x), np.asarray(w))
    res = run_bass_kernel_spmd(nc, in_maps, list(range(NCORES)))
    LAST_RESULTS = res
    return _assemble(res.results)


if __name__ == "__main__":
    # quick single-core CoreSim sanity check of the device program
    from concourse.bass_interp import CoreSim

    rng = np.random.default_rng(0)
    x = rng.normal(size=(B, D)).astype(np.float32)
    w = rng.uniform(-0.05, 0.05, size=(D, UO)).astype(np.float32)

    nc = _build()
    in_maps = _make_inputs(x, w)

    h = (x @ w).reshape(B, U, O)
    diffs = h[:, :, :, None] - np.transpose(h, (1, 2, 0))[None, :, :, :]
    expected = np.exp(-np.abs(diffs).sum(axis=1)).sum(axis=-1)  # [B, O]

    results = []
    for c in range(NCORES):
        sim = CoreSim(nc, trace=False)
        for name, arr in in_maps[c].items():
            sim.tensor(name)[:] = arr
        sim.simulate(check_with_hw=False)
        results.append(
            {"frow": sim.tensor("frow").copy(), "fcol": sim.tensor("fcol").copy()}
        )
        print(f"core {c} simulated")
    got = _assemble(results)
    err = np.abs(got - expected).max() / np.abs(expected).max()
    print("CoreSim rel err vs fp32 numpy reference:", err)
    print(got[:2, :4], expected[:2, :4])


# revision 18
# speedup vs baseline: 1.0096x; 1.0096x over previous
"""Trainium2 Bass kernel for MinibatchDiscrimination.

Reference op:
    h = (x @ w).reshape(B, U, O)                      # B=512, U=32, O=32
    D[i, o, j] = sum_u |h[i,u,o] - h[j,u,o]|          # pairwise L1 over units
    out[i, o]  = sum_j exp(-D[i,o,j])

Strategy (8 NeuronCores, data-parallel over query rows i, half-pair windows):
  - Host: transpose x -> xT [2048, 512], cast x/w to bf16. Each core c gets
    xT rolled so that its own 64 query columns come first; every core sees
    all 512 comparison columns.
  - Each unordered pair is computed once: query i compares against the 256
    columns [i+1, i+256] (mod 512, wrap-free via column-duplicated tiles).
    The diagonal exp(0)=1 is added on the host. Every computed pair (i,j)
    contributes to F[i] via the in-instruction row accumulation and to F[j]
    via a transposed bf16 accumulator F_colT (all its values are < 1e-7, so
    bf16 is ample). Antipodal pairs (distance 256) are computed from both
    ends; their exp is ~1e-20, invisible in fp32.
  - abs-free L1 via |d| = 2*relu(d) - d, distributed over the unit-sum:
        D[o,j] = 2*sum_u Sel*relu(h_j - h_i) - S[o,j] + S[o,i],
    S[o,j] = sum_u h[j,u,o] (computed once by the same selector matmul).
    The -S[o,j] term rides the SAME stationary matrix sel2 as the relu
    chunks via rhs Sq4 (= -S/2 on partitions 0:32, zeros elsewhere), so all
    phase-2 matmuls share one lhsT; followers in each PSUM accumulation
    chain set ldweights=False to skip redundant PE weight loads. +S[o,i] is
    the per-partition bias of the fused ACT exp+accumulate instruction.
  - Elementwise chunks are split between DVE and ACT: DVE chunks use the
    identity |a-b| = 2*max(a,b) - a - b with a single-op
    tensor_scalar(max, per-partition h_i) (fast DVE perf mode); ACT chunks
    (m in ACT_SET) use Relu(h_j - h_i); the exp bias absorbs the
    difference: bias = S_i - 2*S_relu_i. Relu/Exp/Copy share one ACT
    table set, so no table reloads.
  - 4 queries share one PSUM bank via PE column-quadrant matmuls
    (tile_position), so a single ACT instruction does exp+row-accumulate
    for 4 queries at full 128-partition width.
"""

import os
import sys

import numpy as np

for _p in ("/opt/trn_rl_repo", "/root/.axon_site/_ro/trn_rl_repo"):
    if os.path.isdir(_p) and _p not in sys.path:
        sys.path.insert(0, _p)

import ml_dtypes  # noqa: E402

B = 512  # batch
D = 2048  # in features
U = 32  # units
O = 32  # units_out
UO = U * O  # 1024
NCORES = 8
BL = B // NCORES  # 64 own queries per core
W = 256  # comparison window width (half of B)
BD = B + W  # duplicated-column width (wrap-free windows)

KCH = D // 128  # 16 k-chunks
MCH = UO // 128  # 8 uo-chunks

ACT_SET = (6, 7)  # chunks handled by ACT (relu form); the rest go to DVE (max form)
NQ = 4  # queries batched per PSUM bank via PE column-quadrant matmuls
NG = BL // NQ  # 16 quad groups

_CACHE = {}
LAST_RESULTS = None  # BassKernelResults of the most recent run (for profiling)


def _build():
    """Build + compile the (single, SPMD-identical) Bass program."""
    if "nc" in _CACHE:
        return _CACHE["nc"]

    from contextlib import ExitStack

    import concourse.mybir as mybir
    import concourse.tile as tile
    from concourse import bacc

    bf16 = mybir.dt.bfloat16
    f32 = mybir.dt.float32

    nc = bacc.Bacc(
        "TRN2",
        target_bir_lowering=False,
        debug=False,
        enable_asserts=False,
    )

    xt_d = nc.dram_tensor("xt", [D, B], bf16, kind="ExternalInput")
    w_d = nc.dram_tensor("w", [D, UO], bf16, kind="ExternalInput")
    # sel cols 0:32 = Sel1 (p%32==o), 32:64 = Sel2 = 2*Sel1
    sel_d = nc.dram_tensor("sel", [128, 2 * O], bf16, kind="ExternalInput")
    frow_d = nc.dram_tensor("frow", [128, BL // 4], f32, kind="ExternalOutput")
    fcol_d = nc.dram_tensor("fcol", [128, BD], bf16, kind="ExternalOutput")

    with tile.TileContext(nc) as tc, ExitStack() as ctx:
        persist = ctx.enter_context(tc.tile_pool(name="persist", bufs=1))
        a_pool = ctx.enter_context(tc.tile_pool(name="a", bufs=10))
        e_pool = ctx.enter_context(tc.tile_pool(name="e", bufs=4))
        ph_pool = ctx.enter_context(tc.tile_pool(name="ph", bufs=2, space="PSUM"))
        ps_pool = ctx.enter_context(tc.tile_pool(name="ps", bufs=1, space="PSUM"))
        pd_pool = ctx.enter_context(tc.tile_pool(name="pd", bufs=4, space="PSUM"))

        # --- persistent tiles ---
        sel_sb = persist.tile([128, 2 * O], bf16, tag="sel")
        nc.sync.dma_start(sel_sb[:], sel_d[:])
        sel1 = sel_sb[:, 0:O]

        w_sb = []
        xt_sb = []
        for k in range(KCH):
            wt = persist.tile([128, UO], bf16, tag=f"w{k}", name=f"w{k}")
            nc.sync.dma_start(wt[:], w_d[k * 128 : (k + 1) * 128, :])
            w_sb.append(wt)
            xtt = persist.tile([128, B], bf16, tag=f"xt{k}", name=f"xt{k}")
            nc.sync.dma_start(xtt[:], xt_d[k * 128 : (k + 1) * 128, :])
            xt_sb.append(xtt)

        hT = [
            persist.tile([128, BD], bf16, tag=f"hT{m}", name=f"hT{m}")
            for m in range(MCH)
        ]
        # per-chunk per-query scalar columns: -h_i for ACT relu chunks,
        # +h_i for DVE max chunks
        hb = [
            persist.tile([128, BL], f32, tag=f"hb{m}", name=f"hb{m}")
            for m in range(MCH)
        ]
        F4 = persist.tile([128, NG], f32, tag="F4")
        FcolT = persist.tile([128, BD], bf16, tag="FcolT")
        Sq4 = persist.tile([128, BD], bf16, tag="Sq4")
        Ss = persist.tile([O, BL], f32, tag="Ss")
        SrA = persist.tile([O, BL], f32, tag="SrA")
        biasT = persist.tile([O, BL], f32, tag="biasT")
        biasS = persist.tile([128, NG], f32, tag="biasS")
        sel2_t = persist.tile([128, O], bf16, tag="sel2t")
        zero_col = persist.tile([128, 1], f32, tag="zc")

        nc.gpsimd.memset(FcolT[:], 0.0)
        nc.gpsimd.memset(Sq4[:], 0.0)

        # --- phase 1: hT = (x @ w)^T in bf16, chunked over uo ---
        for m in range(MCH):
            ph = ph_pool.tile([128, B], f32)
            for k in range(KCH):
                nc.tensor.matmul(
                    ph[:],
                    w_sb[k][:, m * 128 : (m + 1) * 128],
                    xt_sb[k][:],
                    start=(k == 0),
                    stop=(k == KCH - 1),
                )
            # PSUM -> SBUF as bf16 (Copy is in the exp/relu table set)
            nc.scalar.activation(hT[m][:, 0:B], ph[:], mybir.ActivationFunctionType.Copy)
            # duplicate first W columns for wrap-free windows
            nc.sync.dma_start(hT[m][:, B:BD], hT[m][:, 0:W])
            # f32 scalar columns for this core's own queries, from the
            # bf16-rounded hT: -h_i for ACT relu chunks, +h_i for DVE max
            nc.vector.tensor_scalar_mul(
                hb[m][:], hT[m][:, 0:BL], -1.0 if m in ACT_SET else 1.0
            )

        # --- phase 1b: S[o, j] = sum_u h[j, u, o] once via Sel1, plus the
        # ACT-chunk partial S_relu used by the exp bias ---
        ps_s = ps_pool.tile([O, B], f32, name="ps_s")
        for m in range(MCH):
            nc.tensor.matmul(
                ps_s[:], sel1, hT[m][:, 0:B], start=(m == 0), stop=(m == MCH - 1)
            )
        # Sq4[0:32] = -S/2 (so sel2 x Sq4 contributes -S[o,j]); rows 32:127 zero
        nc.scalar.activation(
            Sq4[0:O, 0:B], ps_s[:], mybir.ActivationFunctionType.Copy, scale=-0.5
        )
        nc.sync.dma_start(Sq4[0:O, B:BD], Sq4[0:O, 0:W])
        nc.vector.tensor_copy(Ss[:], ps_s[:, 0:BL])

        ps_r = ps_pool.tile([O, BL], f32, name="ps_r")
        for n, m in enumerate(ACT_SET):
            nc.tensor.matmul(
                ps_r[:],
                sel1,
                hT[m][:, 0:BL],
                start=(n == 0),
                stop=(n == len(ACT_SET) - 1),
            )
        nc.vector.tensor_copy(SrA[:], ps_r[:])
        # exp bias: D = P - S_i + 2*S_relu_i  =>  bias = S_i - 2*S_relu_i
        nc.vector.tensor_scalar_mul(SrA[:], SrA[:], -2.0)
        nc.vector.tensor_tensor(biasT[:], Ss[:], SrA[:], mybir.AluOpType.add)
        # stack bias columns to the quad layout [32q+o, g] <- [o, 4g+q]
        for q in range(NQ):
            nc.sync.dma_start(biasS[O * q : O * (q + 1), :], biasT[:, q::NQ])

        # Dependency gate: sel2_t is derived through zero_col <- Sq4 <- ps_s
        # <- all S matmuls <- all hT copies <- all h matmuls. Every phase-2
        # matmul reads sel2_t, so no differently-weighted matmul can be
        # scheduled into phase 2 (required for the ldweights=False skips).
        nc.vector.tensor_scalar(
            zero_col[:], Sq4[:, 0:1], 0.0, None, mybir.AluOpType.mult
        )
        nc.vector.tensor_scalar(
            sel2_t[:], sel_sb[:, O : 2 * O], zero_col[:], None, mybir.AluOpType.add
        )

        # --- phase 2: per-query windowed pairwise L1 + exp-sum,
        # 4 queries batched per PSUM bank via PE column quadrants ---
        for g in range(NG):
            pd = pd_pool.tile([128, W], f32)
            for q in range(NQ):
                i = NQ * g + q
                lo = i + 1  # window = local columns [i+1, i+256]
                for m in range(MCH):
                    a = a_pool.tile([128, W], bf16, tag="a")
                    if m in ACT_SET:
                        nc.scalar.activation(
                            a[:],
                            hT[m][:, lo : lo + W],
                            mybir.ActivationFunctionType.Relu,
                            bias=hb[m][:, i : i + 1],
                            scale=1.0,
                        )
                    else:
                        # max(h_j, h_i): |d| = 2*max(a,b) - a - b
                        nc.vector.tensor_scalar(
                            a[:],
                            hT[m][:, lo : lo + W],
                            hb[m][:, i : i + 1],
                            None,
                            mybir.AluOpType.max,
                        )
                    nc.tensor.matmul(
                        pd[O * q : O * (q + 1), :],
                        sel2_t[:],
                        a[:],
                        start=(m == 0),
                        stop=False,
                        tile_position=(0, O * q),
                    )
                nc.tensor.matmul(
                    pd[O * q : O * (q + 1), :],
                    sel2_t[:],
                    Sq4[:, lo : lo + W],
                    start=False,
                    stop=True,
                    tile_position=(0, O * q),
                )

            e = e_pool.tile([128, W], bf16, tag="e")
            nc.scalar.activation(
                e[:],
                pd[:],
                mybir.ActivationFunctionType.Exp,
                bias=biasS[:, g : g + 1],
                scale=-1.0,
                accum_out=F4[:, g : g + 1],
            )
            # transposed-side contributions (tiny values; bf16 is ample)
            for q in range(NQ):
                i = NQ * g + q
                lo = i + 1
                nc.vector.tensor_tensor(
                    FcolT[O * q : O * (q + 1), lo : lo + W],
                    FcolT[O * q : O * (q + 1), lo : lo + W],
                    e[O * q : O * (q + 1), :],
                    mybir.AluOpType.add,
                )

        nc.sync.dma_start(frow_d[:], F4[:])
        nc.sync.dma_start(fcol_d[:], FcolT[:])

    nc.compile()
    _strip_redundant_ldweights(nc)
    _CACHE["nc"] = nc
    return nc


def _strip_redundant_ldweights(nc):
    """Drop PE weight reloads whose weights AP matches the already-loaded one.

    The Tile lowering splits every matmul into Ldweights+Matmult (matmuls all
    carry ldweights=False). Phase 2 issues 64*9 matmuls with the same
    stationary matrix across 4 PE column quadrants; reloading per matmul
    costs ~35us of PE. A reload is removable iff it has no semaphore
    waits/updates and its quadrant (tile_position) already holds the
    identical weights AP; any unrecognized PE instruction conservatively
    invalidates the tracked state.
    """
    import concourse.mybir as mybir

    PE = mybir.EngineType.PE
    keep_state = {"InstMatmult", "InstDrain", "InstEventSemaphore", "InstNop"}
    removed = 0
    for blk in nc.m.functions[0].blocks:
        insts = blk.instructions
        out = []
        loaded = {}  # tile_position -> weights key
        for inst in insts:
            nm = type(inst).__name__
            if nm == "InstLdweights":
                ap = inst.ins[0]
                pos = tuple(inst.tile_position or (0, 0))
                key = (
                    ap.memref,
                    ap.offset,
                    tuple(map(tuple, ap.ap)),
                    str(ap.dtype),
                    inst.is_transpose,
                    inst.perf_mode,
                    tuple(inst.tile_size or ()),
                )
                si = inst.sync_info
                has_sync = si is not None and (
                    list(si.on_wait or []) or list(si.on_update or [])
                )
                if not has_sync and loaded.get(pos) == key:
                    removed += 1
                    continue
                if pos == (0, 0) and (inst.tile_size is None):
                    # full-array load clobbers every quadrant
                    loaded = {}
                loaded[pos] = key
            elif nm not in keep_state and getattr(inst, "engine", None) == PE:
                loaded = {}
            out.append(inst)
        if removed:
            blk.instructions = out
    return removed


def _make_inputs(x: np.ndarray, w: np.ndarray):
    """Host-side prep: transpose/cast/roll into per-core input maps."""
    xt = np.ascontiguousarray(x.T).astype(ml_dtypes.bfloat16)  # [D, B]
    wb = w.astype(ml_dtypes.bfloat16)  # [D, UO]
    sel = np.zeros((128, 2 * O), dtype=ml_dtypes.bfloat16)
    sel[np.arange(128), np.arange(128) % O] = 1
    sel[np.arange(128), O + np.arange(128) % O] = 2
    in_maps = []
    for c in range(NCORES):
        xt_c = np.roll(xt, -BL * c, axis=1)
        in_maps.append({"xt": np.ascontiguousarray(xt_c), "w": wb, "sel": sel})
    return in_maps


def _assemble(results) -> np.ndarray:
    """Host-side gather: diagonal + row accums + transposed col accums."""
    out = np.ones((B, O), dtype=np.float64)
    for c in range(NCORES):
        frow = np.asarray(results[c]["frow"]).astype(np.float64)  # [128, 16]
        # frow[32q + o, g] = row-sum for query i = 4g + q
        fr = frow.reshape(NQ, O, NG)  # [q, o, g]
        rows = fr.transpose(2, 0, 1).reshape(BL, O)  # [i = 4g+q -> (g, q), o]
        out[BL * c : BL * (c + 1), :] += rows
        fcol = np.asarray(results[c]["fcol"]).astype(np.float64)  # [128, BD]
        fc = fcol.reshape(NQ, O, BD).sum(axis=0)  # [o, BD] summed over q groups
        fold = fc[:, :B]
        fold[:, :W] += fc[:, B:BD]
        idx = (np.arange(B) + BL * c) % B
        out[idx, :] += fold.T
    return out.astype(np.float32)


def kernel(x: np.ndarray, w: np.ndarray) -> np.ndarray:
    global LAST_RESULTS
    from concourse.bass_utils import run_bass_kernel_spmd

    nc = _build()
    in_maps = _make_inputs(np.asarray(x), np.asarray(w))
    res = run_bass_kernel_spmd(nc, in_maps, list(range(NCORES)))
    LAST_RESULTS = res
    return _assemble(res.results)


if __name__ == "__main__":
    # quick single-core CoreSim sanity check of the device program
    from concourse.bass_interp import CoreSim

    rng = np.random.default_rng(0)
    x = rng.normal(size=(B, D)).astype(np.float32)
    w = rng.uniform(-0.05, 0.05, size=(D, UO)).astype(np.float32)

    nc = _build()
    in_maps = _make_inputs(x, w)

    h = (x @ w).reshape(B, U, O)
    diffs = h[:, :, :, None] - np.transpose(h, (1, 2, 0))[None, :, :, :]
    expected = np.exp(-np.abs(diffs).sum(axis=1)).sum(axis=-1)  # [B, O]

    results = []
    for c in range(NCORES):
        sim = CoreSim(nc, trace=False)
        for name, arr in in_maps[c].items():
            sim.tensor(name)[:] = arr
        sim.simulate(check_with_hw=False)
        results.append(
            {"frow": sim.tensor("frow").copy(), "fcol": sim.tensor("fcol").copy()}
        )
        print(f"core {c} simulated")
    got = _assemble(results)
    err = np.abs(got - expected).max() / np.abs(expected).max()
    print("CoreSim rel err vs fp32 numpy reference:", err)
    print(got[:2, :4], expected[:2, :4])


# revision 19
# speedup vs baseline: 1.1309x; 1.1202x over previous
"""Trainium2 Bass kernel for MinibatchDiscrimination.

Reference op:
    h = (x @ w).reshape(B, U, O)                      # B=512, U=32, O=32
    D[i, o, j] = sum_u |h[i,u,o] - h[j,u,o]|          # pairwise L1 over units
    out[i, o]  = sum_j exp(-D[i,o,j])

Strategy (8 NeuronCores, data-parallel over query rows i, half-pair windows):
  - Host: transpose x -> xT [2048, 512], cast x/w to bf16. Each core c gets
    xT rolled so that its own 64 query columns come first; every core sees
    all 512 comparison columns.
  - Each unordered pair is computed once: query i compares against the 256
    columns [i+1, i+256] (mod 512, wrap-free via column-duplicated tiles).
    The diagonal exp(0)=1 is added on the host. Every computed pair (i,j)
    contributes to F[i] via the in-instruction row accumulation and to F[j]
    via a transposed bf16 accumulator F_colT (all its values are < 1e-7, so
    bf16 is ample). Antipodal pairs (distance 256) are computed from both
    ends; their exp is ~1e-20, invisible in fp32.
  - abs-free L1 via |d| = 2*relu(d) - d, distributed over the unit-sum:
        D[o,j] = 2*sum_u Sel*relu(h_j - h_i) - S[o,j] + S[o,i],
    S[o,j] = sum_u h[j,u,o] (computed once by the same selector matmul).
    The -S[o,j] term rides the SAME stationary matrix sel2 as the relu
    chunks via rhs Sq4 (= -S/2 on partitions 0:32, zeros elsewhere), so all
    phase-2 matmuls share one lhsT; followers in each PSUM accumulation
    chain set ldweights=False to skip redundant PE weight loads. +S[o,i] is
    the per-partition bias of the fused ACT exp+accumulate instruction.
  - Elementwise chunks are split between DVE and ACT: DVE chunks use the
    identity |a-b| = 2*max(a,b) - a - b with a single-op
    tensor_scalar(max, per-partition h_i) (fast DVE perf mode); ACT chunks
    (m in ACT_SET) use Relu(h_j - h_i); the exp bias absorbs the
    difference: bias = S_i - 2*S_relu_i. Relu/Exp/Copy share one ACT
    table set, so no table reloads.
  - 4 queries share one PSUM bank via PE column-quadrant matmuls
    (tile_position), so a single ACT instruction does exp+row-accumulate
    for 4 queries at full 128-partition width.
"""

import os
import sys

import numpy as np

for _p in ("/opt/trn_rl_repo", "/root/.axon_site/_ro/trn_rl_repo"):
    if os.path.isdir(_p) and _p not in sys.path:
        sys.path.insert(0, _p)

import ml_dtypes  # noqa: E402

B = 512  # batch
D = 2048  # in features
U = 32  # units
O = 32  # units_out
UO = U * O  # 1024
NCORES = 8
BL = B // NCORES  # 64 own queries per core
W = 256  # comparison window width (half of B)
FW = W + BL  # skewed F_col accumulator width (windows end at col 63+256)

KCH = D // 128  # 16 k-chunks
MCH = UO // 128  # 8 uo-chunks

ACT_SET = (6, 7)  # chunks handled by ACT (relu form); the rest go to DVE (max form)
NQ = 4  # queries batched per PSUM bank via PE column-quadrant matmuls
NG = BL // NQ  # 16 quad groups

_CACHE = {}
LAST_RESULTS = None  # BassKernelResults of the most recent run (for profiling)


def _build():
    """Build + compile the (single, SPMD-identical) Bass program."""
    if "nc" in _CACHE:
        return _CACHE["nc"]

    from contextlib import ExitStack

    import concourse.mybir as mybir
    import concourse.tile as tile
    from concourse import bacc

    bf16 = mybir.dt.bfloat16
    f32 = mybir.dt.float32

    nc = bacc.Bacc(
        "TRN2",
        target_bir_lowering=False,
        debug=False,
        enable_asserts=False,
    )

    xt_d = nc.dram_tensor("xt", [D, B], bf16, kind="ExternalInput")
    w_d = nc.dram_tensor("w", [D, UO], bf16, kind="ExternalInput")
    # sel cols 0:32 = Sel1 (p%32==o), 32:64 = Sel2 = 2*Sel1
    sel_d = nc.dram_tensor("sel", [128, 2 * O], bf16, kind="ExternalInput")
    frow_d = nc.dram_tensor("frow", [128, BL // 4], f32, kind="ExternalOutput")
    fcol_d = nc.dram_tensor("fcol", [128, FW], bf16, kind="ExternalOutput")

    with tile.TileContext(nc) as tc, ExitStack() as ctx:
        persist = ctx.enter_context(tc.tile_pool(name="persist", bufs=1))
        a_pool = ctx.enter_context(tc.tile_pool(name="a", bufs=12))
        e_pool = ctx.enter_context(tc.tile_pool(name="e", bufs=4))
        ph_pool = ctx.enter_context(tc.tile_pool(name="ph", bufs=2, space="PSUM"))
        ps_pool = ctx.enter_context(tc.tile_pool(name="ps", bufs=1, space="PSUM"))
        pd_pool = ctx.enter_context(tc.tile_pool(name="pd", bufs=4, space="PSUM"))

        # --- persistent tiles ---
        sel_sb = persist.tile([128, 2 * O], bf16, tag="sel")
        nc.sync.dma_start(sel_sb[:], sel_d[:])
        sel1 = sel_sb[:, 0:O]

        w_sb = []
        xt_sb = []
        for k in range(KCH):
            wt = persist.tile([128, UO], bf16, tag=f"w{k}", name=f"w{k}")
            nc.sync.dma_start(wt[:], w_d[k * 128 : (k + 1) * 128, :])
            w_sb.append(wt)
            xtt = persist.tile([128, B], bf16, tag=f"xt{k}", name=f"xt{k}")
            nc.sync.dma_start(xtt[:], xt_d[k * 128 : (k + 1) * 128, :])
            xt_sb.append(xtt)

        hT = [
            persist.tile([128, B], bf16, tag=f"hT{m}", name=f"hT{m}")
            for m in range(MCH)
        ]
        # per-chunk per-query scalar columns: -h_i for ACT relu chunks,
        # +h_i for DVE max chunks
        hb = [
            persist.tile([128, BL], f32, tag=f"hb{m}", name=f"hb{m}")
            for m in range(MCH)
        ]
        F4 = persist.tile([128, NG], f32, tag="F4")
        FcolT = persist.tile([128, FW], bf16, tag="FcolT")
        Sq4 = persist.tile([128, B], bf16, tag="Sq4")
        Ss = persist.tile([O, BL], f32, tag="Ss")
        SrA = persist.tile([O, BL], f32, tag="SrA")
        biasT = persist.tile([O, BL], f32, tag="biasT")
        biasS = persist.tile([128, NG], f32, tag="biasS")
        sel2_t = persist.tile([128, O], bf16, tag="sel2t")
        zero_col = persist.tile([128, 1], f32, tag="zc")

        nc.gpsimd.memset(FcolT[:], 0.0)
        nc.gpsimd.memset(Sq4[:], 0.0)

        # --- phase 1: hT = (x @ w)^T in bf16, chunked over uo ---
        for m in range(MCH):
            ph = ph_pool.tile([128, B], f32)
            for k in range(KCH):
                nc.tensor.matmul(
                    ph[:],
                    w_sb[k][:, m * 128 : (m + 1) * 128],
                    xt_sb[k][:],
                    start=(k == 0),
                    stop=(k == KCH - 1),
                )
            # PSUM -> SBUF as bf16 (Copy is in the exp/relu table set)
            nc.scalar.activation(hT[m][:, 0:B], ph[:], mybir.ActivationFunctionType.Copy)
            # f32 scalar columns for this core's own queries, from the
            # bf16-rounded hT: -h_i for ACT relu chunks, +h_i for DVE max
            nc.vector.tensor_scalar_mul(
                hb[m][:], hT[m][:, 0:BL], -1.0 if m in ACT_SET else 1.0
            )

        # --- phase 1b: S[o, j] = sum_u h[j, u, o] once via Sel1, plus the
        # ACT-chunk partial S_relu used by the exp bias ---
        ps_s = ps_pool.tile([O, B], f32, name="ps_s")
        for m in range(MCH):
            nc.tensor.matmul(
                ps_s[:], sel1, hT[m][:, 0:B], start=(m == 0), stop=(m == MCH - 1)
            )
        # Sq4[0:32] = -S/2 (so sel2 x Sq4 contributes -S[o,j]); rows 32:127 zero
        nc.scalar.activation(
            Sq4[0:O, 0:B], ps_s[:], mybir.ActivationFunctionType.Copy, scale=-0.5
        )
        nc.vector.tensor_copy(Ss[:], ps_s[:, 0:BL])

        ps_r = ps_pool.tile([O, BL], f32, name="ps_r")
        for n, m in enumerate(ACT_SET):
            nc.tensor.matmul(
                ps_r[:],
                sel1,
                hT[m][:, 0:BL],
                start=(n == 0),
                stop=(n == len(ACT_SET) - 1),
            )
        nc.vector.tensor_copy(SrA[:], ps_r[:])
        # exp bias: D = P - S_i + 2*S_relu_i  =>  bias = S_i - 2*S_relu_i
        nc.vector.tensor_scalar_mul(SrA[:], SrA[:], -2.0)
        nc.vector.tensor_tensor(biasT[:], Ss[:], SrA[:], mybir.AluOpType.add)
        # stack bias columns to the quad layout [32q+o, g] <- [o, 4g+q]
        for q in range(NQ):
            nc.sync.dma_start(biasS[O * q : O * (q + 1), :], biasT[:, q::NQ])

        # Dependency gate: sel2_t is derived through zero_col <- Sq4 <- ps_s
        # <- all S matmuls <- all hT copies <- all h matmuls. Every phase-2
        # matmul reads sel2_t, so no differently-weighted matmul can be
        # scheduled into phase 2 (required for the ldweights=False skips).
        nc.vector.tensor_scalar(
            zero_col[:], Sq4[:, 0:1], 0.0, None, mybir.AluOpType.mult
        )
        nc.vector.tensor_scalar(
            sel2_t[:], sel_sb[:, O : 2 * O], zero_col[:], None, mybir.AluOpType.add
        )

        # --- phase 2: per-query windowed pairwise L1 + exp-sum,
        # 4 queries batched per PSUM bank via PE column quadrants ---
        for g in range(NG):
            pd = pd_pool.tile([128, W], f32)
            for q in range(NQ):
                i = NQ * g + q
                lo = i + 1  # window = local columns [i+1, i+256]
                # the -S[o,j] term first: its rhs is static, so PE can start
                # each chain without waiting on DVE/ACT chunk producers
                nc.tensor.matmul(
                    pd[O * q : O * (q + 1), :],
                    sel2_t[:],
                    Sq4[:, lo : lo + W],
                    start=True,
                    stop=False,
                    tile_position=(0, O * q),
                )
                for m in range(MCH):
                    a = a_pool.tile([128, W], bf16, tag="a")
                    if m in ACT_SET:
                        nc.scalar.activation(
                            a[:],
                            hT[m][:, lo : lo + W],
                            mybir.ActivationFunctionType.Relu,
                            bias=hb[m][:, i : i + 1],
                            scale=1.0,
                        )
                    else:
                        # max(h_j, h_i): |d| = 2*max(a,b) - a - b
                        nc.vector.tensor_scalar(
                            a[:],
                            hT[m][:, lo : lo + W],
                            hb[m][:, i : i + 1],
                            None,
                            mybir.AluOpType.max,
                        )
                    nc.tensor.matmul(
                        pd[O * q : O * (q + 1), :],
                        sel2_t[:],
                        a[:],
                        start=False,
                        stop=(m == MCH - 1),
                        tile_position=(0, O * q),
                    )

            e = e_pool.tile([128, W], bf16, tag="e")
            nc.scalar.activation(
                e[:],
                pd[:],
                mybir.ActivationFunctionType.Exp,
                bias=biasS[:, g : g + 1],
                scale=-1.0,
                accum_out=F4[:, g : g + 1],
            )
            # transposed-side contributions (tiny values; bf16 is ample).
            # FcolT is SKEWED: row 32q+o column L holds the contribution to
            # local column L+q, so the whole quad is one tensor add.
            nc.vector.tensor_tensor(
                FcolT[:, NQ * g + 1 : NQ * g + 1 + W],
                FcolT[:, NQ * g + 1 : NQ * g + 1 + W],
                e[:],
                mybir.AluOpType.add,
            )

        nc.sync.dma_start(frow_d[:], F4[:])
        nc.sync.dma_start(fcol_d[:], FcolT[:])

    nc.compile()
    _strip_redundant_ldweights(nc)
    _CACHE["nc"] = nc
    return nc


def _strip_redundant_ldweights(nc):
    """Drop PE weight reloads whose weights AP matches the already-loaded one.

    The Tile lowering splits every matmul into Ldweights+Matmult (matmuls all
    carry ldweights=False). Phase 2 issues 64*9 matmuls with the same
    stationary matrix across 4 PE column quadrants; reloading per matmul
    costs ~35us of PE. A reload is removable iff it has no semaphore
    waits/updates and its quadrant (tile_position) already holds the
    identical weights AP; any unrecognized PE instruction conservatively
    invalidates the tracked state.
    """
    import concourse.mybir as mybir

    PE = mybir.EngineType.PE
    keep_state = {"InstMatmult", "InstDrain", "InstEventSemaphore", "InstNop"}
    removed = 0
    for blk in nc.m.functions[0].blocks:
        insts = blk.instructions
        out = []
        loaded = {}  # tile_position -> weights key
        for inst in insts:
            nm = type(inst).__name__
            if nm == "InstLdweights":
                ap = inst.ins[0]
                pos = tuple(inst.tile_position or (0, 0))
                key = (
                    ap.memref,
                    ap.offset,
                    tuple(map(tuple, ap.ap)),
                    str(ap.dtype),
                    inst.is_transpose,
                    inst.perf_mode,
                    tuple(inst.tile_size or ()),
                )
                si = inst.sync_info
                has_sync = si is not None and (
                    list(si.on_wait or []) or list(si.on_update or [])
                )
                if not has_sync and loaded.get(pos) == key:
                    removed += 1
                    continue
                if pos == (0, 0) and (inst.tile_size is None):
                    # full-array load clobbers every quadrant
                    loaded = {}
                loaded[pos] = key
            elif nm not in keep_state and getattr(inst, "engine", None) == PE:
                loaded = {}
            out.append(inst)
        if removed:
            blk.instructions = out
    return removed


def _make_inputs(x: np.ndarray, w: np.ndarray):
    """Host-side prep: transpose/cast/roll into per-core input maps."""
    xt = np.ascontiguousarray(x.T).astype(ml_dtypes.bfloat16)  # [D, B]
    wb = w.astype(ml_dtypes.bfloat16)  # [D, UO]
    sel = np.zeros((128, 2 * O), dtype=ml_dtypes.bfloat16)
    sel[np.arange(128), np.arange(128) % O] = 1
    sel[np.arange(128), O + np.arange(128) % O] = 2
    in_maps = []
    for c in range(NCORES):
        xt_c = np.roll(xt, -BL * c, axis=1)
        in_maps.append({"xt": np.ascontiguousarray(xt_c), "w": wb, "sel": sel})
    return in_maps


def _assemble(results) -> np.ndarray:
    """Host-side gather: diagonal + row accums + transposed col accums."""
    out = np.ones((B, O), dtype=np.float64)
    for c in range(NCORES):
        frow = np.asarray(results[c]["frow"]).astype(np.float64)  # [128, 16]
        # frow[32q + o, g] = row-sum for query i = 4g + q
        fr = frow.reshape(NQ, O, NG)  # [q, o, g]
        rows = fr.transpose(2, 0, 1).reshape(BL, O)  # [i = 4g+q -> (g, q), o]
        out[BL * c : BL * (c + 1), :] += rows
        fcol = np.asarray(results[c]["fcol"]).astype(np.float64)  # [128, FW]
        # unskew: row 32q+o column L -> local column L + q
        fc = fcol.reshape(NQ, O, FW)
        fold = np.zeros((O, B), dtype=np.float64)
        for q in range(NQ):
            fold[:, q : q + FW] += fc[q]
        idx = (np.arange(B) + BL * c) % B
        out[idx, :] += fold.T
    return out.astype(np.float32)


def kernel(x: np.ndarray, w: np.ndarray) -> np.ndarray:
    global LAST_RESULTS
    from concourse.bass_utils import run_bass_kernel_spmd

    nc = _build()
    in_maps = _make_inputs(np.asarray(x), np.asarray(w))
    res = run_bass_kernel_spmd(nc, in_maps, list(range(NCORES)))
    LAST_RESULTS = res
    return _assemble(res.results)


if __name__ == "__main__":
    # quick single-core CoreSim sanity check of the device program
    from concourse.bass_interp import CoreSim

    rng = np.random.default_rng(0)
    x = rng.normal(size=(B, D)).astype(np.float32)
    w = rng.uniform(-0.05, 0.05, size=(D, UO)).astype(np.float32)

    nc = _build()
    in_maps = _make_inputs(x, w)

    h = (x @ w).reshape(B, U, O)
    diffs = h[:, :, :, None] - np.transpose(h, (1, 2, 0))[None, :, :, :]
    expected = np.exp(-np.abs(diffs).sum(axis=1)).sum(axis=-1)  # [B, O]

    results = []
    for c in range(NCORES):
        sim = CoreSim(nc, trace=False)
        for name, arr in in_maps[c].items():
            sim.tensor(name)[:] = arr
        sim.simulate(check_with_hw=False)
        results.append(
            {"frow": sim.tensor("frow").copy(), "fcol": sim.tensor("fcol").copy()}
        )
        print(f"core {c} simulated")
    got = _assemble(results)
    err = np.abs(got - expected).max() / np.abs(expected).max()
    print("CoreSim rel err vs fp32 numpy reference:", err)
    print(got[:2, :4], expected[:2, :4])


# revision 20
# speedup vs baseline: 1.1489x; 1.0159x over previous
"""Trainium2 Bass kernel for MinibatchDiscrimination.

Reference op:
    h = (x @ w).reshape(B, U, O)                      # B=512, U=32, O=32
    D[i, o, j] = sum_u |h[i,u,o] - h[j,u,o]|          # pairwise L1 over units
    out[i, o]  = sum_j exp(-D[i,o,j])

Strategy (8 NeuronCores, data-parallel over query rows i, half-pair windows):
  - Host: transpose x -> xT [2048, 512], cast x/w to bf16. Each core c gets
    xT rolled so that its own 64 query columns come first; every core sees
    all 512 comparison columns.
  - Each unordered pair is computed once: query i compares against the 256
    columns [i+1, i+256] (mod 512, wrap-free via column-duplicated tiles).
    The diagonal exp(0)=1 is added on the host. Every computed pair (i,j)
    contributes to F[i] via the in-instruction row accumulation and to F[j]
    via a transposed bf16 accumulator F_colT (all its values are < 1e-7, so
    bf16 is ample). Antipodal pairs (distance 256) are computed from both
    ends; their exp is ~1e-20, invisible in fp32.
  - abs-free L1 via |d| = 2*relu(d) - d, distributed over the unit-sum:
        D[o,j] = 2*sum_u Sel*relu(h_j - h_i) - S[o,j] + S[o,i],
    S[o,j] = sum_u h[j,u,o] (computed once by the same selector matmul).
    The -S[o,j] term rides the SAME stationary matrix sel2 as the relu
    chunks via rhs Sq4 (= -S/2 on partitions 0:32, zeros elsewhere), so all
    phase-2 matmuls share one lhsT; followers in each PSUM accumulation
    chain set ldweights=False to skip redundant PE weight loads. +S[o,i] is
    the per-partition bias of the fused ACT exp+accumulate instruction.
  - Elementwise chunks are split between DVE and ACT: DVE chunks use the
    identity |a-b| = 2*max(a,b) - a - b with a single-op
    tensor_scalar(max, per-partition h_i) (fast DVE perf mode); ACT chunks
    (m in ACT_SET) use Relu(h_j - h_i); the exp bias absorbs the
    difference: bias = S_i - 2*S_relu_i. Relu/Exp/Copy share one ACT
    table set, so no table reloads.
  - 4 queries share one PSUM bank via PE column-quadrant matmuls
    (tile_position), so a single ACT instruction does exp+row-accumulate
    for 4 queries at full 128-partition width.
"""

import os
import sys

import numpy as np

for _p in ("/opt/trn_rl_repo", "/root/.axon_site/_ro/trn_rl_repo"):
    if os.path.isdir(_p) and _p not in sys.path:
        sys.path.insert(0, _p)

import ml_dtypes  # noqa: E402

B = 512  # batch
D = 2048  # in features
U = 32  # units
O = 32  # units_out
UO = U * O  # 1024
NCORES = 8
BL = B // NCORES  # 64 own queries per core
W = 256  # comparison window width (half of B)
FW = W + BL  # skewed F_col accumulator width (windows end at col 63+256)

KCH = D // 128  # 16 k-chunks
MCH = UO // 128  # 8 uo-chunks

ACT_SET = (6, 7)  # chunks handled by ACT (relu form); the rest go to DVE (max form)
NQ = 4  # queries batched per PSUM bank via PE column-quadrant matmuls
NG = BL // NQ  # 16 quad groups

_CACHE = {}
LAST_RESULTS = None  # BassKernelResults of the most recent run (for profiling)


def _build():
    """Build + compile the (single, SPMD-identical) Bass program."""
    if "nc" in _CACHE:
        return _CACHE["nc"]

    from contextlib import ExitStack

    import concourse.mybir as mybir
    import concourse.tile as tile
    from concourse import bacc

    bf16 = mybir.dt.bfloat16
    f32 = mybir.dt.float32

    nc = bacc.Bacc(
        "TRN2",
        target_bir_lowering=False,
        debug=False,
        enable_asserts=False,
    )

    xt_d = nc.dram_tensor("xt", [D, B], bf16, kind="ExternalInput")
    w_d = nc.dram_tensor("w", [D, UO], bf16, kind="ExternalInput")
    # sel cols 0:32 = Sel1 (p%32==o), 32:64 = Sel2 = 2*Sel1
    sel_d = nc.dram_tensor("sel", [128, 2 * O], bf16, kind="ExternalInput")
    frow_d = nc.dram_tensor("frow", [128, BL // 4], f32, kind="ExternalOutput")
    fcol_d = nc.dram_tensor("fcol", [128, FW], bf16, kind="ExternalOutput")

    with tile.TileContext(nc) as tc, ExitStack() as ctx:
        persist = ctx.enter_context(tc.tile_pool(name="persist", bufs=1))
        a_pool = ctx.enter_context(tc.tile_pool(name="a", bufs=12))
        e_pool = ctx.enter_context(tc.tile_pool(name="e", bufs=4))
        ph_pool = ctx.enter_context(tc.tile_pool(name="ph", bufs=2, space="PSUM"))
        ps_pool = ctx.enter_context(tc.tile_pool(name="ps", bufs=1, space="PSUM"))
        pd_pool = ctx.enter_context(tc.tile_pool(name="pd", bufs=4, space="PSUM"))

        # --- persistent tiles ---
        sel_sb = persist.tile([128, 2 * O], bf16, tag="sel")
        nc.sync.dma_start(sel_sb[:], sel_d[:])
        sel1 = sel_sb[:, 0:O]

        w_sb = []
        xt_sb = []
        for k in range(KCH):
            wt = persist.tile([128, UO], bf16, tag=f"w{k}", name=f"w{k}")
            nc.sync.dma_start(wt[:], w_d[k * 128 : (k + 1) * 128, :])
            w_sb.append(wt)
            xtt = persist.tile([128, B], bf16, tag=f"xt{k}", name=f"xt{k}")
            nc.sync.dma_start(xtt[:], xt_d[k * 128 : (k + 1) * 128, :])
            xt_sb.append(xtt)

        hT = [
            persist.tile([128, B], bf16, tag=f"hT{m}", name=f"hT{m}")
            for m in range(MCH)
        ]
        # per-chunk per-query scalar columns: -h_i for ACT relu chunks,
        # +h_i for DVE max chunks
        hb = [
            persist.tile([128, BL], f32, tag=f"hb{m}", name=f"hb{m}")
            for m in range(MCH)
        ]
        F4 = persist.tile([128, NG], f32, tag="F4")
        FcolT = persist.tile([128, FW], bf16, tag="FcolT")
        Sq4 = persist.tile([128, B], bf16, tag="Sq4")
        Ss = persist.tile([O, BL], f32, tag="Ss")
        SrA = persist.tile([O, BL], f32, tag="SrA")
        biasT = persist.tile([O, BL], f32, tag="biasT")
        biasS = persist.tile([128, NG], f32, tag="biasS")
        sel2_t = persist.tile([128, O], bf16, tag="sel2t")
        zero_col = persist.tile([128, 1], f32, tag="zc")

        nc.gpsimd.memset(FcolT[:], 0.0)
        nc.gpsimd.memset(Sq4[:], 0.0)

        # --- phase 1: hT = (x @ w)^T in bf16, chunked over uo ---
        for m in range(MCH):
            ph = ph_pool.tile([128, B], f32)
            for k in range(KCH):
                nc.tensor.matmul(
                    ph[:],
                    w_sb[k][:, m * 128 : (m + 1) * 128],
                    xt_sb[k][:],
                    start=(k == 0),
                    stop=(k == KCH - 1),
                )
            # PSUM -> SBUF as bf16 (Copy is in the exp/relu table set)
            nc.scalar.activation(hT[m][:, 0:B], ph[:], mybir.ActivationFunctionType.Copy)
            # f32 scalar columns for this core's own queries, from the
            # bf16-rounded hT: -h_i for ACT relu chunks, +h_i for DVE max
            nc.vector.tensor_scalar_mul(
                hb[m][:], hT[m][:, 0:BL], -1.0 if m in ACT_SET else 1.0
            )

        # --- phase 1b: S[o, j] = sum_u h[j, u, o] once via Sel1, plus the
        # ACT-chunk partial S_relu used by the exp bias ---
        ps_s = ps_pool.tile([O, B], f32, name="ps_s")
        for m in range(MCH):
            nc.tensor.matmul(
                ps_s[:], sel1, hT[m][:, 0:B], start=(m == 0), stop=(m == MCH - 1)
            )
        # Sq4[0:32] = -S/2 (so sel2 x Sq4 contributes -S[o,j]); rows 32:127 zero
        nc.scalar.activation(
            Sq4[0:O, 0:B], ps_s[:], mybir.ActivationFunctionType.Copy, scale=-0.5
        )
        nc.vector.tensor_copy(Ss[:], ps_s[:, 0:BL])

        ps_r = ps_pool.tile([O, BL], f32, name="ps_r")
        for n, m in enumerate(ACT_SET):
            nc.tensor.matmul(
                ps_r[:],
                sel1,
                hT[m][:, 0:BL],
                start=(n == 0),
                stop=(n == len(ACT_SET) - 1),
            )
        nc.vector.tensor_copy(SrA[:], ps_r[:])
        # exp bias: D = P - S_i + 2*S_relu_i  =>  bias = S_i - 2*S_relu_i
        nc.vector.tensor_scalar_mul(SrA[:], SrA[:], -2.0)
        nc.vector.tensor_tensor(biasT[:], Ss[:], SrA[:], mybir.AluOpType.add)
        # stack bias columns to the quad layout [32q+o, g] <- [o, 4g+q]
        for q in range(NQ):
            nc.sync.dma_start(biasS[O * q : O * (q + 1), :], biasT[:, q::NQ])

        # Dependency gate: sel2_t is derived through zero_col <- Sq4 <- ps_s
        # <- all S matmuls <- all hT copies <- all h matmuls. Every phase-2
        # matmul reads sel2_t, so no differently-weighted matmul can be
        # scheduled into phase 2 (required for the ldweights=False skips).
        nc.vector.tensor_scalar(
            zero_col[:], Sq4[:, 0:1], 0.0, None, mybir.AluOpType.mult
        )
        nc.vector.tensor_scalar(
            sel2_t[:], sel_sb[:, O : 2 * O], zero_col[:], None, mybir.AluOpType.add
        )

        # --- phase 2: per-query windowed pairwise L1 + exp-sum,
        # 4 queries batched per PSUM bank via PE column quadrants.
        # Emission is software-pipelined: the exp for quad g is emitted after
        # quad g+1's chunk work and the FcolT add after quad g+2's, so those
        # cross-engine-dependent instructions never block the ACT/DVE FIFOs.
        pd_tiles = {}
        e_tiles = {}

        def emit_quad(g):
            pd = pd_pool.tile([128, W], f32, name=f"pd{g}", tag="pd")
            pd_tiles[g] = pd
            for q in range(NQ):
                i = NQ * g + q
                lo = i + 1  # window = local columns [i+1, i+256]
                # the -S[o,j] term first: its rhs is static, so PE can start
                # each chain without waiting on DVE/ACT chunk producers
                nc.tensor.matmul(
                    pd[O * q : O * (q + 1), :],
                    sel2_t[:],
                    Sq4[:, lo : lo + W],
                    start=True,
                    stop=False,
                    tile_position=(0, O * q),
                )
                for m in range(MCH):
                    a = a_pool.tile([128, W], bf16, tag="a", name=f"a{g}_{q}_{m}")
                    if m in ACT_SET:
                        nc.scalar.activation(
                            a[:],
                            hT[m][:, lo : lo + W],
                            mybir.ActivationFunctionType.Relu,
                            bias=hb[m][:, i : i + 1],
                            scale=1.0,
                        )
                    else:
                        # max(h_j, h_i): |d| = 2*max(a,b) - a - b
                        nc.vector.tensor_scalar(
                            a[:],
                            hT[m][:, lo : lo + W],
                            hb[m][:, i : i + 1],
                            None,
                            mybir.AluOpType.max,
                        )
                    nc.tensor.matmul(
                        pd[O * q : O * (q + 1), :],
                        sel2_t[:],
                        a[:],
                        start=False,
                        stop=(m == MCH - 1),
                        tile_position=(0, O * q),
                    )

        def emit_exp(g):
            pd = pd_tiles.pop(g)
            e = e_pool.tile([128, W], bf16, tag="e", name=f"e{g}")
            e_tiles[g] = e
            nc.scalar.activation(
                e[:],
                pd[:],
                mybir.ActivationFunctionType.Exp,
                bias=biasS[:, g : g + 1],
                scale=-1.0,
                accum_out=F4[:, g : g + 1],
            )

        def emit_fcol(g):
            # transposed-side contributions (tiny values; bf16 is ample).
            # FcolT is SKEWED: row 32q+o column L holds the contribution to
            # local column L+q, so the whole quad is one tensor add.
            e = e_tiles.pop(g)
            nc.vector.tensor_tensor(
                FcolT[:, NQ * g + 1 : NQ * g + 1 + W],
                FcolT[:, NQ * g + 1 : NQ * g + 1 + W],
                e[:],
                mybir.AluOpType.add,
            )

        for g in range(NG):
            emit_quad(g)
            if g >= 1:
                emit_exp(g - 1)
            if g >= 2:
                emit_fcol(g - 2)
        emit_exp(NG - 1)
        emit_fcol(NG - 2)
        emit_fcol(NG - 1)

        nc.sync.dma_start(frow_d[:], F4[:])
        nc.sync.dma_start(fcol_d[:], FcolT[:])

    nc.compile()
    _strip_redundant_ldweights(nc)
    _CACHE["nc"] = nc
    return nc


def _strip_redundant_ldweights(nc):
    """Drop PE weight reloads whose weights AP matches the already-loaded one.

    The Tile lowering splits every matmul into Ldweights+Matmult (matmuls all
    carry ldweights=False). Phase 2 issues 64*9 matmuls with the same
    stationary matrix across 4 PE column quadrants; reloading per matmul
    costs ~35us of PE. A reload is removable iff it has no semaphore
    waits/updates and its quadrant (tile_position) already holds the
    identical weights AP; any unrecognized PE instruction conservatively
    invalidates the tracked state.
    """
    import concourse.mybir as mybir

    PE = mybir.EngineType.PE
    keep_state = {"InstMatmult", "InstDrain", "InstEventSemaphore", "InstNop"}
    removed = 0
    for blk in nc.m.functions[0].blocks:
        insts = blk.instructions
        out = []
        loaded = {}  # tile_position -> weights key
        for inst in insts:
            nm = type(inst).__name__
            if nm == "InstLdweights":
                ap = inst.ins[0]
                pos = tuple(inst.tile_position or (0, 0))
                key = (
                    ap.memref,
                    ap.offset,
                    tuple(map(tuple, ap.ap)),
                    str(ap.dtype),
                    inst.is_transpose,
                    inst.perf_mode,
                    tuple(inst.tile_size or ()),
                )
                si = inst.sync_info
                has_sync = si is not None and (
                    list(si.on_wait or []) or list(si.on_update or [])
                )
                if not has_sync and loaded.get(pos) == key:
                    removed += 1
                    continue
                if pos == (0, 0) and (inst.tile_size is None):
                    # full-array load clobbers every quadrant
                    loaded = {}
                loaded[pos] = key
            elif nm not in keep_state and getattr(inst, "engine", None) == PE:
                loaded = {}
            out.append(inst)
        if removed:
            blk.instructions = out
    return removed


def _make_inputs(x: np.ndarray, w: np.ndarray):
    """Host-side prep: transpose/cast/roll into per-core input maps."""
    xt = np.ascontiguousarray(x.T).astype(ml_dtypes.bfloat16)  # [D, B]
    wb = w.astype(ml_dtypes.bfloat16)  # [D, UO]
    sel = np.zeros((128, 2 * O), dtype=ml_dtypes.bfloat16)
    sel[np.arange(128), np.arange(128) % O] = 1
    sel[np.arange(128), O + np.arange(128) % O] = 2
    in_maps = []
    for c in range(NCORES):
        xt_c = np.roll(xt, -BL * c, axis=1)
        in_maps.append({"xt": np.ascontiguousarray(xt_c), "w": wb, "sel": sel})
    return in_maps


def _assemble(results) -> np.ndarray:
    """Host-side gather: diagonal + row accums + transposed col accums."""
    out = np.ones((B, O), dtype=np.float64)
    for c in range(NCORES):
        frow = np.asarray(results[c]["frow"]).astype(np.float64)  # [128, 16]
        # frow[32q + o, g] = row-sum for query i = 4g + q
        fr = frow.reshape(NQ, O, NG)  # [q, o, g]
        rows = fr.transpose(2, 0, 1).reshape(BL, O)  # [i = 4g+q -> (g, q), o]
        out[BL * c : BL * (c + 1), :] += rows
        fcol = np.asarray(results[c]["fcol"]).astype(np.float64)  # [128, FW]
        # unskew: row 32q+o column L -> local column L + q
        fc = fcol.reshape(NQ, O, FW)
        fold = np.zeros((O, B), dtype=np.float64)
        for q in range(NQ):
            fold[:, q : q + FW] += fc[q]
        idx = (np.arange(B) + BL * c) % B
        out[idx, :] += fold.T
    return out.astype(np.float32)


def kernel(x: np.ndarray, w: np.ndarray) -> np.ndarray:
    global LAST_RESULTS
    from concourse.bass_utils import run_bass_kernel_spmd

    nc = _build()
    in_maps = _make_inputs(np.asarray(x), np.asarray(w))
    res = run_bass_kernel_spmd(nc, in_maps, list(range(NCORES)))
    LAST_RESULTS = res
    return _assemble(res.results)


if __name__ == "__main__":
    # quick single-core CoreSim sanity check of the device program
    from concourse.bass_interp import CoreSim

    rng = np.random.default_rng(0)
    x = rng.normal(size=(B, D)).astype(np.float32)
    w = rng.uniform(-0.05, 0.05, size=(D, UO)).astype(np.float32)

    nc = _build()
    in_maps = _make_inputs(x, w)

    h = (x @ w).reshape(B, U, O)
    diffs = h[:, :, :, None] - np.transpose(h, (1, 2, 0))[None, :, :, :]
    expected = np.exp(-np.abs(diffs).sum(axis=1)).sum(axis=-1)  # [B, O]

    results = []
    for c in range(NCORES):
        sim = CoreSim(nc, trace=False)
        for name, arr in in_maps[c].items():
            sim.tensor(name)[:] = arr
        sim.simulate(check_with_hw=False)
        results.append(
            {"frow": sim.tensor("frow").copy(), "fcol": sim.tensor("fcol").copy()}
        )
        print(f"core {c} simulated")
    got = _assemble(results)
    err = np.abs(got - expected).max() / np.abs(expected).max()
    print("CoreSim rel err vs fp32 numpy reference:", err)
    print(got[:2, :4], expected[:2, :4])


# revision 22
# speedup vs baseline: 1.1827x; 1.0294x over previous
"""Trainium2 Bass kernel for MinibatchDiscrimination.

Reference op:
    h = (x @ w).reshape(B, U, O)                      # B=512, U=32, O=32
    D[i, o, j] = sum_u |h[i,u,o] - h[j,u,o]|          # pairwise L1 over units
    out[i, o]  = sum_j exp(-D[i,o,j])

Strategy (8 NeuronCores, data-parallel over query rows i, half-pair windows):
  - Host: transpose x -> xT [2048, 512], cast x/w to bf16. Each core c gets
    xT rolled so that its own 64 query columns come first; every core sees
    all 512 comparison columns.
  - Each unordered pair is computed once: query i compares against the 256
    columns [i+1, i+256] (mod 512, wrap-free via column-duplicated tiles).
    The diagonal exp(0)=1 is added on the host. Every computed pair (i,j)
    contributes to F[i] via the in-instruction row accumulation and to F[j]
    via a transposed bf16 accumulator F_colT (all its values are < 1e-7, so
    bf16 is ample). Antipodal pairs (distance 256) are computed from both
    ends; their exp is ~1e-20, invisible in fp32.
  - abs-free L1 via |d| = 2*relu(d) - d, distributed over the unit-sum:
        D[o,j] = 2*sum_u Sel*relu(h_j - h_i) - S[o,j] + S[o,i],
    S[o,j] = sum_u h[j,u,o] (computed once by the same selector matmul).
    The -S[o,j] term rides the SAME stationary matrix sel2 as the relu
    chunks via rhs Sq4 (= -S/2 on partitions 0:32, zeros elsewhere), so all
    phase-2 matmuls share one lhsT; followers in each PSUM accumulation
    chain set ldweights=False to skip redundant PE weight loads. +S[o,i] is
    the per-partition bias of the fused ACT exp+accumulate instruction.
  - Elementwise chunks are split between DVE and ACT: DVE chunks use the
    identity |a-b| = 2*max(a,b) - a - b with a single-op
    tensor_scalar(max, per-partition h_i) (fast DVE perf mode); ACT chunks
    (m in ACT_SET) use Relu(h_j - h_i); the exp bias absorbs the
    difference: bias = S_i - 2*S_relu_i. Relu/Exp/Copy share one ACT
    table set, so no table reloads.
  - 4 queries share one PSUM bank via PE column-quadrant matmuls
    (tile_position), so a single ACT instruction does exp+row-accumulate
    for 4 queries at full 128-partition width.
"""

import os
import sys

import numpy as np

for _p in ("/opt/trn_rl_repo", "/root/.axon_site/_ro/trn_rl_repo"):
    if os.path.isdir(_p) and _p not in sys.path:
        sys.path.insert(0, _p)

import ml_dtypes  # noqa: E402

B = 512  # batch
D = 2048  # in features
U = 32  # units
O = 32  # units_out
UO = U * O  # 1024
NCORES = 8
BL = B // NCORES  # 64 own queries per core
W = 256  # comparison window width (half of B)
FW = W + BL  # skewed F_col accumulator width (windows end at col 63+256)

KCH = D // 128  # 16 k-chunks
MCH = UO // 128  # 8 uo-chunks

ACT_SET = (6, 7)  # chunks handled by ACT (relu form); the rest go to DVE (max form)
NQ = 4  # queries batched per PSUM bank via PE column-quadrant matmuls
NG = BL // NQ  # 16 quad groups

_CACHE = {}
LAST_RESULTS = None  # BassKernelResults of the most recent run (for profiling)


def _build_h():
    """Launch-1 program: core c computes hT rows [128c, 128c+128) in bf16."""
    if "nc_h" in _CACHE:
        return _CACHE["nc_h"]

    from contextlib import ExitStack

    import concourse.mybir as mybir
    import concourse.tile as tile
    from concourse import bacc

    bf16 = mybir.dt.bfloat16
    f32 = mybir.dt.float32

    nc = bacc.Bacc(
        "TRN2", target_bir_lowering=False, debug=False, enable_asserts=False
    )
    xt_d = nc.dram_tensor("xt", [D, B], bf16, kind="ExternalInput")
    ws_d = nc.dram_tensor("ws", [D, 128], bf16, kind="ExternalInput")
    hts_d = nc.dram_tensor("hts", [128, B], bf16, kind="ExternalOutput")

    with tile.TileContext(nc) as tc, ExitStack() as ctx:
        pool = ctx.enter_context(tc.tile_pool(name="p", bufs=1))
        psum = ctx.enter_context(tc.tile_pool(name="ps", bufs=1, space="PSUM"))
        tiles = []
        for k in range(KCH):
            xtt = pool.tile([128, B], bf16, tag=f"xt{k}", name=f"xt{k}")
            nc.sync.dma_start(xtt[:], xt_d[k * 128 : (k + 1) * 128, :])
            wst = pool.tile([128, 128], bf16, tag=f"ws{k}", name=f"ws{k}")
            nc.sync.dma_start(wst[:], ws_d[k * 128 : (k + 1) * 128, :])
            tiles.append((wst, xtt))
        ph = psum.tile([128, B], f32)
        for k, (wst, xtt) in enumerate(tiles):
            nc.tensor.matmul(
                ph[:], wst[:], xtt[:], start=(k == 0), stop=(k == KCH - 1)
            )
        hts = pool.tile([128, B], bf16, tag="hts")
        nc.scalar.activation(hts[:], ph[:], mybir.ActivationFunctionType.Copy)
        nc.sync.dma_start(hts_d[:], hts[:])

    nc.compile()
    _CACHE["nc_h"] = nc
    return nc


def _build():
    """Build + compile the (single, SPMD-identical) Bass program."""
    if "nc" in _CACHE:
        return _CACHE["nc"]

    from contextlib import ExitStack

    import concourse.mybir as mybir
    import concourse.tile as tile
    from concourse import bacc

    bf16 = mybir.dt.bfloat16
    f32 = mybir.dt.float32

    nc = bacc.Bacc(
        "TRN2",
        target_bir_lowering=False,
        debug=False,
        enable_asserts=False,
    )

    ht_d = nc.dram_tensor("ht", [UO, B], bf16, kind="ExternalInput")
    # sel cols 0:32 = Sel1 (p%32==o), 32:64 = Sel2 = 2*Sel1
    sel_d = nc.dram_tensor("sel", [128, 2 * O], bf16, kind="ExternalInput")
    frow_d = nc.dram_tensor("frow", [128, BL // 4], f32, kind="ExternalOutput")
    fcol_d = nc.dram_tensor("fcol", [128, FW], bf16, kind="ExternalOutput")

    with tile.TileContext(nc) as tc, ExitStack() as ctx:
        persist = ctx.enter_context(tc.tile_pool(name="persist", bufs=1))
        a_pool = ctx.enter_context(tc.tile_pool(name="a", bufs=12))
        e_pool = ctx.enter_context(tc.tile_pool(name="e", bufs=4))
        ps_pool = ctx.enter_context(tc.tile_pool(name="ps", bufs=1, space="PSUM"))
        pd_pool = ctx.enter_context(tc.tile_pool(name="pd", bufs=5, space="PSUM"))

        # --- persistent tiles ---
        sel_sb = persist.tile([128, 2 * O], bf16, tag="sel")
        nc.sync.dma_start(sel_sb[:], sel_d[:])
        sel1 = sel_sb[:, 0:O]

        hT = [
            persist.tile([128, B], bf16, tag=f"hT{m}", name=f"hT{m}")
            for m in range(MCH)
        ]
        # per-chunk per-query scalar columns: -h_i for ACT relu chunks,
        # +h_i for DVE max chunks
        hb = [
            persist.tile([128, BL], f32, tag=f"hb{m}", name=f"hb{m}")
            for m in range(MCH)
        ]
        F4 = persist.tile([128, NG], f32, tag="F4")
        FcolT = persist.tile([128, FW], bf16, tag="FcolT")
        Sq4 = persist.tile([128, B], bf16, tag="Sq4")
        Ss = persist.tile([O, BL], f32, tag="Ss")
        SrA = persist.tile([O, BL], f32, tag="SrA")
        biasT = persist.tile([O, BL], f32, tag="biasT")
        biasS = persist.tile([128, NG], f32, tag="biasS")
        sel2_t = persist.tile([128, O], bf16, tag="sel2t")
        zero_col = persist.tile([128, 1], f32, tag="zc")

        nc.gpsimd.memset(FcolT[:], 0.0)
        nc.gpsimd.memset(Sq4[:], 0.0)

        # --- phase 1: load hT (computed by the launch-1 program) ---
        for m in range(MCH):
            nc.sync.dma_start(hT[m][:], ht_d[m * 128 : (m + 1) * 128, :])
            # f32 scalar columns for this core's own queries, from the
            # bf16-rounded hT: -h_i for ACT relu chunks, +h_i for DVE max
            nc.vector.tensor_scalar_mul(
                hb[m][:], hT[m][:, 0:BL], -1.0 if m in ACT_SET else 1.0
            )

        # --- phase 1b: S[o, j] = sum_u h[j, u, o] once via Sel1, plus the
        # ACT-chunk partial S_relu used by the exp bias ---
        ps_s = ps_pool.tile([O, B], f32, name="ps_s")
        for m in range(MCH):
            nc.tensor.matmul(
                ps_s[:], sel1, hT[m][:, 0:B], start=(m == 0), stop=(m == MCH - 1)
            )
        # Sq4[0:32] = -S/2 (so sel2 x Sq4 contributes -S[o,j]); rows 32:127 zero
        nc.scalar.activation(
            Sq4[0:O, 0:B], ps_s[:], mybir.ActivationFunctionType.Copy, scale=-0.5
        )
        nc.vector.tensor_copy(Ss[:], ps_s[:, 0:BL])

        ps_r = ps_pool.tile([O, BL], f32, name="ps_r")
        for n, m in enumerate(ACT_SET):
            nc.tensor.matmul(
                ps_r[:],
                sel1,
                hT[m][:, 0:BL],
                start=(n == 0),
                stop=(n == len(ACT_SET) - 1),
            )
        nc.vector.tensor_copy(SrA[:], ps_r[:])
        # exp bias: D = P - S_i + 2*S_relu_i  =>  bias = S_i - 2*S_relu_i
        nc.vector.tensor_scalar_mul(SrA[:], SrA[:], -2.0)
        nc.vector.tensor_tensor(biasT[:], Ss[:], SrA[:], mybir.AluOpType.add)
        # stack bias columns to the quad layout [32q+o, g] <- [o, 4g+q]
        for q in range(NQ):
            nc.sync.dma_start(biasS[O * q : O * (q + 1), :], biasT[:, q::NQ])

        # Dependency gate: sel2_t is derived through zero_col <- Sq4 <- ps_s
        # <- all S matmuls <- all hT copies <- all h matmuls. Every phase-2
        # matmul reads sel2_t, so no differently-weighted matmul can be
        # scheduled into phase 2 (required for the ldweights=False skips).
        nc.vector.tensor_scalar(
            zero_col[:], Sq4[:, 0:1], 0.0, None, mybir.AluOpType.mult
        )
        nc.vector.tensor_scalar(
            sel2_t[:], sel_sb[:, O : 2 * O], zero_col[:], None, mybir.AluOpType.add
        )

        # --- phase 2: per-query windowed pairwise L1 + exp-sum,
        # 4 queries batched per PSUM bank via PE column quadrants.
        # Emission is software-pipelined: the exp for quad g is emitted after
        # quad g+1's chunk work and the FcolT add after quad g+2's, so those
        # cross-engine-dependent instructions never block the ACT/DVE FIFOs.
        pd_tiles = {}
        e_tiles = {}

        def emit_quad(g):
            pd = pd_pool.tile([128, W], f32, name=f"pd{g}", tag="pd")
            pd_tiles[g] = pd
            for q in range(NQ):
                i = NQ * g + q
                lo = i + 1  # window = local columns [i+1, i+256]
                # the -S[o,j] term first: its rhs is static, so PE can start
                # each chain without waiting on DVE/ACT chunk producers
                nc.tensor.matmul(
                    pd[O * q : O * (q + 1), :],
                    sel2_t[:],
                    Sq4[:, lo : lo + W],
                    start=True,
                    stop=False,
                    tile_position=(0, O * q),
                )
                for m in range(MCH):
                    a = a_pool.tile([128, W], bf16, tag="a", name=f"a{g}_{q}_{m}")
                    if m in ACT_SET:
                        nc.scalar.activation(
                            a[:],
                            hT[m][:, lo : lo + W],
                            mybir.ActivationFunctionType.Relu,
                            bias=hb[m][:, i : i + 1],
                            scale=1.0,
                        )
                    else:
                        # max(h_j, h_i): |d| = 2*max(a,b) - a - b
                        nc.vector.tensor_scalar(
                            a[:],
                            hT[m][:, lo : lo + W],
                            hb[m][:, i : i + 1],
                            None,
                            mybir.AluOpType.max,
                        )
                    nc.tensor.matmul(
                        pd[O * q : O * (q + 1), :],
                        sel2_t[:],
                        a[:],
                        start=False,
                        stop=(m == MCH - 1),
                        tile_position=(0, O * q),
                    )

        def emit_exp(g):
            pd = pd_tiles.pop(g)
            e = e_pool.tile([128, W], bf16, tag="e", name=f"e{g}")
            e_tiles[g] = e
            nc.scalar.activation(
                e[:],
                pd[:],
                mybir.ActivationFunctionType.Exp,
                bias=biasS[:, g : g + 1],
                scale=-1.0,
                accum_out=F4[:, g : g + 1],
            )

        def emit_fcol(g):
            # transposed-side contributions (tiny values; bf16 is ample).
            # FcolT is SKEWED: row 32q+o column L holds the contribution to
            # local column L+q, so the whole quad is one tensor add.
            e = e_tiles.pop(g)
            nc.vector.tensor_tensor(
                FcolT[:, NQ * g + 1 : NQ * g + 1 + W],
                FcolT[:, NQ * g + 1 : NQ * g + 1 + W],
                e[:],
                mybir.AluOpType.add,
            )

        for g in range(NG):
            emit_quad(g)
            if g >= 1:
                emit_exp(g - 1)
            if g >= 2:
                emit_fcol(g - 2)
        emit_exp(NG - 1)
        emit_fcol(NG - 2)
        emit_fcol(NG - 1)

        nc.sync.dma_start(frow_d[:], F4[:])
        nc.sync.dma_start(fcol_d[:], FcolT[:])

    nc.compile()
    _strip_redundant_ldweights(nc)
    _CACHE["nc"] = nc
    return nc


def _strip_redundant_ldweights(nc):
    """Drop PE weight reloads whose weights AP matches the already-loaded one.

    The Tile lowering splits every matmul into Ldweights+Matmult (matmuls all
    carry ldweights=False). Phase 2 issues 64*9 matmuls with the same
    stationary matrix across 4 PE column quadrants; reloading per matmul
    costs ~35us of PE. A reload is removable iff it has no semaphore
    waits/updates and its quadrant (tile_position) already holds the
    identical weights AP; any unrecognized PE instruction conservatively
    invalidates the tracked state.
    """
    import concourse.mybir as mybir

    PE = mybir.EngineType.PE
    keep_state = {"InstMatmult", "InstDrain", "InstEventSemaphore", "InstNop"}
    removed = 0
    for blk in nc.m.functions[0].blocks:
        insts = blk.instructions
        out = []
        loaded = {}  # tile_position -> weights key
        for inst in insts:
            nm = type(inst).__name__
            if nm == "InstLdweights":
                ap = inst.ins[0]
                pos = tuple(inst.tile_position or (0, 0))
                key = (
                    ap.memref,
                    ap.offset,
                    tuple(map(tuple, ap.ap)),
                    str(ap.dtype),
                    inst.is_transpose,
                    inst.perf_mode,
                    tuple(inst.tile_size or ()),
                )
                si = inst.sync_info
                has_sync = si is not None and (
                    list(si.on_wait or []) or list(si.on_update or [])
                )
                if not has_sync and loaded.get(pos) == key:
                    removed += 1
                    continue
                if pos == (0, 0) and (inst.tile_size is None):
                    # full-array load clobbers every quadrant
                    loaded = {}
                loaded[pos] = key
            elif nm not in keep_state and getattr(inst, "engine", None) == PE:
                loaded = {}
            out.append(inst)
        if removed:
            blk.instructions = out
    return removed


def _make_inputs_h(x: np.ndarray, w: np.ndarray):
    xt = np.ascontiguousarray(x.T).astype(ml_dtypes.bfloat16)  # [D, B]
    wb = w.astype(ml_dtypes.bfloat16)  # [D, UO]
    return [
        {"xt": xt, "ws": np.ascontiguousarray(wb[:, 128 * c : 128 * (c + 1)])}
        for c in range(NCORES)
    ]


def _make_inputs_main(ht_global: np.ndarray):
    sel = np.zeros((128, 2 * O), dtype=ml_dtypes.bfloat16)
    sel[np.arange(128), np.arange(128) % O] = 1
    sel[np.arange(128), O + np.arange(128) % O] = 2
    return [
        {"ht": np.ascontiguousarray(np.roll(ht_global, -BL * c, axis=1)), "sel": sel}
        for c in range(NCORES)
    ]


def _assemble(results) -> np.ndarray:
    """Host-side gather: diagonal + row accums + transposed col accums."""
    out = np.ones((B, O), dtype=np.float64)
    for c in range(NCORES):
        frow = np.asarray(results[c]["frow"]).astype(np.float64)  # [128, 16]
        # frow[32q + o, g] = row-sum for query i = 4g + q
        fr = frow.reshape(NQ, O, NG)  # [q, o, g]
        rows = fr.transpose(2, 0, 1).reshape(BL, O)  # [i = 4g+q -> (g, q), o]
        out[BL * c : BL * (c + 1), :] += rows
        fcol = np.asarray(results[c]["fcol"]).astype(np.float64)  # [128, FW]
        # unskew: row 32q+o column L -> local column L + q
        fc = fcol.reshape(NQ, O, FW)
        fold = np.zeros((O, B), dtype=np.float64)
        for q in range(NQ):
            fold[:, q : q + FW] += fc[q]
        idx = (np.arange(B) + BL * c) % B
        out[idx, :] += fold.T
    return out.astype(np.float32)


def kernel(x: np.ndarray, w: np.ndarray) -> np.ndarray:
    global LAST_RESULTS
    from concourse.bass_utils import run_bass_kernel_spmd

    nc_h = _build_h()
    nc = _build()
    res_h = run_bass_kernel_spmd(
        nc_h, _make_inputs_h(np.asarray(x), np.asarray(w)), list(range(NCORES))
    )
    ht_global = np.concatenate(
        [np.asarray(res_h.results[c]["hts"]) for c in range(NCORES)], axis=0
    )
    res = run_bass_kernel_spmd(nc, _make_inputs_main(ht_global), list(range(NCORES)))
    LAST_RESULTS = (res_h, res)
    return _assemble(res.results)


if __name__ == "__main__":
    # quick CoreSim sanity check of both device programs
    from concourse.bass_interp import CoreSim

    rng = np.random.default_rng(0)
    x = rng.normal(size=(B, D)).astype(np.float32)
    w = rng.uniform(-0.05, 0.05, size=(D, UO)).astype(np.float32)

    nc_h = _build_h()
    nc = _build()

    hts = []
    for c, im in enumerate(_make_inputs_h(x, w)):
        sim = CoreSim(nc_h, trace=False)
        for name, arr in im.items():
            sim.tensor(name)[:] = arr
        sim.simulate(check_with_hw=False)
        hts.append(sim.tensor("hts").copy())
    ht_global = np.concatenate(hts, axis=0)
    print("launch-1 simulated")

    h = (x @ w).reshape(B, U, O)
    diffs = h[:, :, :, None] - np.transpose(h, (1, 2, 0))[None, :, :, :]
    expected = np.exp(-np.abs(diffs).sum(axis=1)).sum(axis=-1)  # [B, O]

    results = []
    for c, im in enumerate(_make_inputs_main(ht_global)):
        sim = CoreSim(nc, trace=False)
        for name, arr in im.items():
            sim.tensor(name)[:] = arr
        sim.simulate(check_with_hw=False)
        results.append(
            {"frow": sim.tensor("frow").copy(), "fcol": sim.tensor("fcol").copy()}
        )
        print(f"core {c} simulated")
    got = _assemble(results)
    err = np.abs(got - expected).max() / np.abs(expected).max()
    print("CoreSim rel err vs fp32 numpy reference:", err)
    print(got[:2, :4], expected[:2, :4])


# revision 24
# speedup vs baseline: 1.2568x; 1.0627x over previous
"""Trainium2 Bass kernel for MinibatchDiscrimination.

Reference op:
    h = (x @ w).reshape(B, U, O)                      # B=512, U=32, O=32
    D[i, o, j] = sum_u |h[i,u,o] - h[j,u,o]|          # pairwise L1 over units
    out[i, o]  = sum_j exp(-D[i,o,j])

Strategy (8 NeuronCores, data-parallel over query rows i, half-pair windows):
  - Host: transpose x -> xT [2048, 512], cast x/w to bf16. Each core c gets
    xT rolled so that its own 64 query columns come first; every core sees
    all 512 comparison columns.
  - Each unordered pair is computed once: query i compares against the 256
    columns [i+1, i+256] (mod 512, wrap-free via column-duplicated tiles).
    The diagonal exp(0)=1 is added on the host. Every computed pair (i,j)
    contributes to F[i] via the in-instruction row accumulation and to F[j]
    via a transposed bf16 accumulator F_colT (all its values are < 1e-7, so
    bf16 is ample). Antipodal pairs (distance 256) are computed from both
    ends; their exp is ~1e-20, invisible in fp32.
  - abs-free L1 via |d| = 2*relu(d) - d, distributed over the unit-sum:
        D[o,j] = 2*sum_u Sel*relu(h_j - h_i) - S[o,j] + S[o,i],
    S[o,j] = sum_u h[j,u,o] (computed once by the same selector matmul).
    The -S[o,j] term rides the SAME stationary matrix sel2 as the relu
    chunks via rhs Sq4 (= -S/2 on partitions 0:32, zeros elsewhere), so all
    phase-2 matmuls share one lhsT; followers in each PSUM accumulation
    chain set ldweights=False to skip redundant PE weight loads. +S[o,i] is
    the per-partition bias of the fused ACT exp+accumulate instruction.
  - Elementwise chunks are split between DVE and ACT: DVE chunks use the
    identity |a-b| = 2*max(a,b) - a - b with a single-op
    tensor_scalar(max, per-partition h_i) (fast DVE perf mode); ACT chunks
    (m in ACT_SET) use Relu(h_j - h_i); the exp bias absorbs the
    difference: bias = S_i - 2*S_relu_i. Relu/Exp/Copy share one ACT
    table set, so no table reloads.
  - 4 queries share one PSUM bank via PE column-quadrant matmuls
    (tile_position), so a single ACT instruction does exp+row-accumulate
    for 4 queries at full 128-partition width.
"""

import os
import sys

import numpy as np

for _p in ("/opt/trn_rl_repo", "/root/.axon_site/_ro/trn_rl_repo"):
    if os.path.isdir(_p) and _p not in sys.path:
        sys.path.insert(0, _p)

import ml_dtypes  # noqa: E402

B = 512  # batch
D = 2048  # in features
U = 32  # units
O = 32  # units_out
UO = U * O  # 1024
NCORES = 8
BL = B // NCORES  # 64 own queries per core
W = 256  # comparison window width (half of B)
FW = W + BL  # skewed F_col accumulator width (windows end at col 63+256)

KCH = D // 128  # 16 k-chunks
MCH = UO // 128  # 8 uo-chunks

ACT_SET = (6, 7)  # chunks handled by ACT (relu form); the rest go to DVE (max form)
NQ = 4  # queries batched per PSUM bank via PE column-quadrant matmuls
NG = BL // NQ  # 16 quad groups

_CACHE = {}
LAST_RESULTS = None  # BassKernelResults of the most recent run (for profiling)


def _build_h():
    """Launch-1 program: core c computes hT rows [128c, 128c+128) in bf16."""
    if "nc_h" in _CACHE:
        return _CACHE["nc_h"]

    from contextlib import ExitStack

    import concourse.mybir as mybir
    import concourse.tile as tile
    from concourse import bacc

    bf16 = mybir.dt.bfloat16
    f32 = mybir.dt.float32

    nc = bacc.Bacc(
        "TRN2", target_bir_lowering=False, debug=False, enable_asserts=False
    )
    xt_d = nc.dram_tensor("xt", [D, B], bf16, kind="ExternalInput")
    ws_d = nc.dram_tensor("ws", [D, 128], bf16, kind="ExternalInput")
    hts_d = nc.dram_tensor("hts", [128, B], bf16, kind="ExternalOutput")

    with tile.TileContext(nc) as tc, ExitStack() as ctx:
        pool = ctx.enter_context(tc.tile_pool(name="p", bufs=1))
        psum = ctx.enter_context(tc.tile_pool(name="ps", bufs=1, space="PSUM"))
        # single strided DMAs: dst [128, k*W] <- DRAM [k*128 + p, :]
        xt_sb = pool.tile([128, KCH * B], bf16, tag="xt")
        nc.sync.dma_start(
            xt_sb.rearrange("p (k j) -> p k j", k=KCH),
            xt_d.rearrange("(k p) j -> p k j", k=KCH),
        )
        ws_sb = pool.tile([128, KCH * 128], bf16, tag="ws")
        nc.sync.dma_start(
            ws_sb.rearrange("p (k j) -> p k j", k=KCH),
            ws_d.rearrange("(k p) j -> p k j", k=KCH),
        )
        ph = psum.tile([128, B], f32)
        for k in range(KCH):
            nc.tensor.matmul(
                ph[:],
                ws_sb[:, k * 128 : (k + 1) * 128],
                xt_sb[:, k * B : (k + 1) * B],
                start=(k == 0),
                stop=(k == KCH - 1),
            )
        hts = pool.tile([128, B], bf16, tag="hts")
        nc.scalar.activation(hts[:], ph[:], mybir.ActivationFunctionType.Copy)
        nc.sync.dma_start(hts_d[:], hts[:])

    nc.compile()
    _CACHE["nc_h"] = nc
    return nc


def _build():
    """Build + compile the (single, SPMD-identical) Bass program."""
    if "nc" in _CACHE:
        return _CACHE["nc"]

    from contextlib import ExitStack

    import concourse.mybir as mybir
    import concourse.tile as tile
    from concourse import bacc

    bf16 = mybir.dt.bfloat16
    f32 = mybir.dt.float32

    nc = bacc.Bacc(
        "TRN2",
        target_bir_lowering=False,
        debug=False,
        enable_asserts=False,
    )

    ht_d = nc.dram_tensor("ht", [UO, B], bf16, kind="ExternalInput")
    # sel cols 0:32 = Sel1 (p%32==o), 32:64 = Sel2 = 2*Sel1
    sel_d = nc.dram_tensor("sel", [128, 2 * O], bf16, kind="ExternalInput")
    frow_d = nc.dram_tensor("frow", [128, BL // 4], f32, kind="ExternalOutput")
    fcol_d = nc.dram_tensor("fcol", [128, FW], bf16, kind="ExternalOutput")

    with tile.TileContext(nc) as tc, ExitStack() as ctx:
        persist = ctx.enter_context(tc.tile_pool(name="persist", bufs=1))
        a_pool = ctx.enter_context(tc.tile_pool(name="a", bufs=12))
        e_pool = ctx.enter_context(tc.tile_pool(name="e", bufs=4))
        ps_pool = ctx.enter_context(tc.tile_pool(name="ps", bufs=1, space="PSUM"))
        pd_pool = ctx.enter_context(tc.tile_pool(name="pd", bufs=5, space="PSUM"))

        # --- persistent tiles ---
        sel_sb = persist.tile([128, 2 * O], bf16, tag="sel")
        nc.sync.dma_start(sel_sb[:], sel_d[:])
        sel1 = sel_sb[:, 0:O]

        # per-chunk per-query scalar columns: -h_i for ACT relu chunks,
        # +h_i for DVE max chunks
        hb = [
            persist.tile([128, BL], f32, tag=f"hb{m}", name=f"hb{m}")
            for m in range(MCH)
        ]
        hb5n = persist.tile([128, BL], f32, tag="hb5n")
        F4 = persist.tile([128, NG], f32, tag="F4")
        FcolT = persist.tile([128, FW], bf16, tag="FcolT")
        Sq4 = persist.tile([128, B], bf16, tag="Sq4")
        Ss = persist.tile([O, BL], f32, tag="Ss")
        SrA = persist.tile([O, BL], f32, tag="SrA")
        biasT = persist.tile([O, BL], f32, tag="biasT")
        S5 = persist.tile([O, BL], f32, tag="S5")
        biasS = persist.tile([128, NG], f32, tag="biasS")
        sel2_t = persist.tile([128, O], bf16, tag="sel2t")
        zero_col = persist.tile([128, 1], f32, tag="zc")

        nc.gpsimd.memset(FcolT[:], 0.0)
        nc.gpsimd.memset(Sq4[:], 0.0)

        # --- phase 1: load hT (computed by the launch-1 program) ---
        hT_all = persist.tile([128, MCH * B], bf16, tag="hT_all")
        nc.sync.dma_start(
            hT_all.rearrange("p (m j) -> p m j", m=MCH),
            ht_d.rearrange("(m p) j -> p m j", m=MCH),
        )
        hT = [hT_all[:, m * B : (m + 1) * B] for m in range(MCH)]
        for m in range(MCH):
            # f32 scalar columns for this core's own queries, from the
            # bf16-rounded hT: -h_i for ACT relu chunks, +h_i for DVE max
            nc.vector.tensor_scalar_mul(
                hb[m][:], hT[m][:, 0:BL], -1.0 if m in ACT_SET else 1.0
            )
        # chunk 5 goes to ACT (relu form, negative bias) for every 4th query
        nc.vector.tensor_scalar_mul(hb5n[:], hT[5][:, 0:BL], -1.0)

        # --- phase 1b: S[o, j] = sum_u h[j, u, o] once via Sel1, plus the
        # ACT-chunk partial S_relu used by the exp bias ---
        ps_s = ps_pool.tile([O, B], f32, name="ps_s")
        for m in range(MCH):
            nc.tensor.matmul(
                ps_s[:], sel1, hT[m][:, 0:B], start=(m == 0), stop=(m == MCH - 1)
            )
        # Sq4[0:32] = -S/2 (so sel2 x Sq4 contributes -S[o,j]); rows 32:127 zero
        nc.scalar.activation(
            Sq4[0:O, 0:B], ps_s[:], mybir.ActivationFunctionType.Copy, scale=-0.5
        )
        nc.vector.tensor_copy(Ss[:], ps_s[:, 0:BL])

        ps_r = ps_pool.tile([O, BL], f32, name="ps_r")
        for n, m in enumerate(ACT_SET):
            nc.tensor.matmul(
                ps_r[:],
                sel1,
                hT[m][:, 0:BL],
                start=(n == 0),
                stop=(n == len(ACT_SET) - 1),
            )
        nc.vector.tensor_copy(SrA[:], ps_r[:])
        ps_r5 = ps_pool.tile([O, BL], f32, name="ps_r5", tag="ps_r")
        nc.tensor.matmul(ps_r5[:], sel1, hT[5][:, 0:BL], start=True, stop=True)
        nc.vector.tensor_copy(S5[:], ps_r5[:])
        # exp bias: D = P - S_i + 2*S_relu_i  =>  bias = S_i - 2*S_relu_i
        nc.vector.tensor_scalar_mul(SrA[:], SrA[:], -2.0)
        nc.vector.tensor_tensor(biasT[:], Ss[:], SrA[:], mybir.AluOpType.add)
        # queries with i%4==3 also run chunk 5 on ACT in relu form
        nc.vector.tensor_scalar_mul(S5[:], S5[:], -2.0)
        nc.vector.tensor_tensor(
            biasT[:, 3::NQ], biasT[:, 3::NQ], S5[:, 3::NQ], mybir.AluOpType.add
        )
        # stack bias columns to the quad layout [32q+o, g] <- [o, 4g+q]
        for q in range(NQ):
            nc.sync.dma_start(biasS[O * q : O * (q + 1), :], biasT[:, q::NQ])

        # Dependency gate: sel2_t is derived through zero_col <- Sq4 <- ps_s
        # <- all S matmuls <- all hT copies <- all h matmuls. Every phase-2
        # matmul reads sel2_t, so no differently-weighted matmul can be
        # scheduled into phase 2 (required for the ldweights=False skips).
        nc.vector.tensor_scalar(
            zero_col[:], Sq4[:, 0:1], 0.0, None, mybir.AluOpType.mult
        )
        nc.vector.tensor_scalar(
            sel2_t[:], sel_sb[:, O : 2 * O], zero_col[:], None, mybir.AluOpType.add
        )

        # --- phase 2: per-query windowed pairwise L1 + exp-sum,
        # 4 queries batched per PSUM bank via PE column quadrants.
        # Emission is software-pipelined: the exp for quad g is emitted after
        # quad g+1's chunk work and the FcolT add after quad g+2's, so those
        # cross-engine-dependent instructions never block the ACT/DVE FIFOs.
        pd_tiles = {}
        e_tiles = {}

        def emit_quad(g):
            pd = pd_pool.tile([128, W], f32, name=f"pd{g}", tag="pd")
            pd_tiles[g] = pd
            for q in range(NQ):
                i = NQ * g + q
                lo = i + 1  # window = local columns [i+1, i+256]
                # the -S[o,j] term first: its rhs is static, so PE can start
                # each chain without waiting on DVE/ACT chunk producers
                nc.tensor.matmul(
                    pd[O * q : O * (q + 1), :],
                    sel2_t[:],
                    Sq4[:, lo : lo + W],
                    start=True,
                    stop=False,
                    tile_position=(0, O * q),
                )
                for m in range(MCH):
                    a = a_pool.tile([128, W], bf16, tag="a", name=f"a{g}_{q}_{m}")
                    if m == 5 and q == 3:
                        nc.scalar.activation(
                            a[:],
                            hT[m][:, lo : lo + W],
                            mybir.ActivationFunctionType.Relu,
                            bias=hb5n[:, i : i + 1],
                            scale=1.0,
                        )
                    elif m in ACT_SET:
                        nc.scalar.activation(
                            a[:],
                            hT[m][:, lo : lo + W],
                            mybir.ActivationFunctionType.Relu,
                            bias=hb[m][:, i : i + 1],
                            scale=1.0,
                        )
                    else:
                        # max(h_j, h_i): |d| = 2*max(a,b) - a - b
                        nc.vector.tensor_scalar(
                            a[:],
                            hT[m][:, lo : lo + W],
                            hb[m][:, i : i + 1],
                            None,
                            mybir.AluOpType.max,
                        )
                    nc.tensor.matmul(
                        pd[O * q : O * (q + 1), :],
                        sel2_t[:],
                        a[:],
                        start=False,
                        stop=(m == MCH - 1),
                        tile_position=(0, O * q),
                    )

        def emit_exp(g):
            pd = pd_tiles.pop(g)
            e = e_pool.tile([128, W], bf16, tag="e", name=f"e{g}")
            e_tiles[g] = e
            nc.scalar.activation(
                e[:],
                pd[:],
                mybir.ActivationFunctionType.Exp,
                bias=biasS[:, g : g + 1],
                scale=-1.0,
                accum_out=F4[:, g : g + 1],
            )

        def emit_fcol(g):
            # transposed-side contributions (tiny values; bf16 is ample).
            # FcolT is SKEWED: row 32q+o column L holds the contribution to
            # local column L+q, so the whole quad is one tensor add.
            e = e_tiles.pop(g)
            nc.vector.tensor_tensor(
                FcolT[:, NQ * g + 1 : NQ * g + 1 + W],
                FcolT[:, NQ * g + 1 : NQ * g + 1 + W],
                e[:],
                mybir.AluOpType.add,
            )

        for g in range(NG):
            emit_quad(g)
            if g >= 1:
                emit_exp(g - 1)
            if g >= 2:
                emit_fcol(g - 2)
        emit_exp(NG - 1)
        emit_fcol(NG - 2)
        emit_fcol(NG - 1)

        nc.sync.dma_start(frow_d[:], F4[:])
        nc.sync.dma_start(fcol_d[:], FcolT[:])

    nc.compile()
    _strip_redundant_ldweights(nc)
    _CACHE["nc"] = nc
    return nc


def _strip_redundant_ldweights(nc):
    """Drop PE weight reloads whose weights AP matches the already-loaded one.

    The Tile lowering splits every matmul into Ldweights+Matmult (matmuls all
    carry ldweights=False). Phase 2 issues 64*9 matmuls with the same
    stationary matrix across 4 PE column quadrants; reloading per matmul
    costs ~35us of PE. A reload is removable iff it has no semaphore
    waits/updates and its quadrant (tile_position) already holds the
    identical weights AP; any unrecognized PE instruction conservatively
    invalidates the tracked state.
    """
    import concourse.mybir as mybir

    PE = mybir.EngineType.PE
    keep_state = {"InstMatmult", "InstDrain", "InstEventSemaphore", "InstNop"}
    removed = 0
    for blk in nc.m.functions[0].blocks:
        insts = blk.instructions
        out = []
        loaded = {}  # tile_position -> weights key
        for inst in insts:
            nm = type(inst).__name__
            if nm == "InstLdweights":
                ap = inst.ins[0]
                pos = tuple(inst.tile_position or (0, 0))
                key = (
                    ap.memref,
                    ap.offset,
                    tuple(map(tuple, ap.ap)),
                    str(ap.dtype),
                    inst.is_transpose,
                    inst.perf_mode,
                    tuple(inst.tile_size or ()),
                )
                si = inst.sync_info
                has_sync = si is not None and (
                    list(si.on_wait or []) or list(si.on_update or [])
                )
                if not has_sync and loaded.get(pos) == key:
                    removed += 1
                    continue
                if pos == (0, 0) and (inst.tile_size is None):
                    # full-array load clobbers every quadrant
                    loaded = {}
                loaded[pos] = key
            elif nm not in keep_state and getattr(inst, "engine", None) == PE:
                loaded = {}
            out.append(inst)
        if removed:
            blk.instructions = out
    return removed


def _make_inputs_h(x: np.ndarray, w: np.ndarray):
    xt = np.ascontiguousarray(x.T).astype(ml_dtypes.bfloat16)  # [D, B]
    wb = w.astype(ml_dtypes.bfloat16)  # [D, UO]
    return [
        {"xt": xt, "ws": np.ascontiguousarray(wb[:, 128 * c : 128 * (c + 1)])}
        for c in range(NCORES)
    ]


def _make_inputs_main(ht_global: np.ndarray):
    sel = np.zeros((128, 2 * O), dtype=ml_dtypes.bfloat16)
    sel[np.arange(128), np.arange(128) % O] = 1
    sel[np.arange(128), O + np.arange(128) % O] = 2
    return [
        {"ht": np.ascontiguousarray(np.roll(ht_global, -BL * c, axis=1)), "sel": sel}
        for c in range(NCORES)
    ]


def _assemble(results) -> np.ndarray:
    """Host-side gather: diagonal + row accums + transposed col accums."""
    out = np.ones((B, O), dtype=np.float64)
    for c in range(NCORES):
        frow = np.asarray(results[c]["frow"]).astype(np.float64)  # [128, 16]
        # frow[32q + o, g] = row-sum for query i = 4g + q
        fr = frow.reshape(NQ, O, NG)  # [q, o, g]
        rows = fr.transpose(2, 0, 1).reshape(BL, O)  # [i = 4g+q -> (g, q), o]
        out[BL * c : BL * (c + 1), :] += rows
        fcol = np.asarray(results[c]["fcol"]).astype(np.float64)  # [128, FW]
        # unskew: row 32q+o column L -> local column L + q
        fc = fcol.reshape(NQ, O, FW)
        fold = np.zeros((O, B), dtype=np.float64)
        for q in range(NQ):
            fold[:, q : q + FW] += fc[q]
        idx = (np.arange(B) + BL * c) % B
        out[idx, :] += fold.T
    return out.astype(np.float32)


def kernel(x: np.ndarray, w: np.ndarray) -> np.ndarray:
    global LAST_RESULTS
    from concourse.bass_utils import run_bass_kernel_spmd

    nc_h = _build_h()
    nc = _build()
    res_h = run_bass_kernel_spmd(
        nc_h, _make_inputs_h(np.asarray(x), np.asarray(w)), list(range(NCORES))
    )
    ht_global = np.concatenate(
        [np.asarray(res_h.results[c]["hts"]) for c in range(NCORES)], axis=0
    )
    res = run_bass_kernel_spmd(nc, _make_inputs_main(ht_global), list(range(NCORES)))
    LAST_RESULTS = (res_h, res)
    return _assemble(res.results)


if __name__ == "__main__":
    # quick CoreSim sanity check of both device programs
    from concourse.bass_interp import CoreSim

    rng = np.random.default_rng(0)
    x = rng.normal(size=(B, D)).astype(np.float32)
    w = rng.uniform(-0.05, 0.05, size=(D, UO)).astype(np.float32)

    nc_h = _build_h()
    nc = _build()

    hts = []
    for c, im in enumerate(_make_inputs_h(x, w)):
        sim = CoreSim(nc_h, trace=False)
        for name, arr in im.items():
            sim.tensor(name)[:] = arr
        sim.simulate(check_with_hw=False)
        hts.append(sim.tensor("hts").copy())
    ht_global = np.concatenate(hts, axis=0)
    print("launch-1 simulated")

    h = (x @ w).reshape(B, U, O)
    diffs = h[:, :, :, None] - np.transpose(h, (1, 2, 0))[None, :, :, :]
    expected = np.exp(-np.abs(diffs).sum(axis=1)).sum(axis=-1)  # [B, O]

    results = []
    for c, im in enumerate(_make_inputs_main(ht_global)):
        sim = CoreSim(nc, trace=False)
        for name, arr in im.items():
            sim.tensor(name)[:] = arr
        sim.simulate(check_with_hw=False)
        results.append(
            {"frow": sim.tensor("frow").copy(), "fcol": sim.tensor("fcol").copy()}
        )
        print(f"core {c} simulated")
    got = _assemble(results)
    err = np.abs(got - expected).max() / np.abs(expected).max()
    print("CoreSim rel err vs fp32 numpy reference:", err)
    print(got[:2, :4], expected[:2, :4])
